# revision 8
# baseline (speedup 1.0000x reference)
"""ALPHA scattering-covariance kernel for 8 Trainium2 NeuronCores.

Math (per batch element b, nc=1, M=N=128, J=5 L=4 A=4, P=960 pairs + 1 phi):
  hatx = fft2(x)
  z_jl = ifft2(hatx * hatpsi[j,l])           (20 complex fields)
  field k=4*jl+a: u = relu(phase_a(z_jl)),   phase in {Re, -Im, -Re, +Im}
  phi field (k=80): u = Re(ifft2(hatx*hatphi))   (no relu)
  n_k = (u - mean)/std ;  hat_n_k = fft2(n_k)
  pair p: corr = Re(ifft2(hat_n[la1] * conj(hat_n[la2]))) / (M*N) * masks[shifted]
  out[b] = concat(pairs 0..959, phi autocorr)

2D FFTs run as transpose-matmuls on the TensorEngine: tmm(P, Q) := P^T @ Q with
P stationary; fft2(n) = W n W (W symmetric), two chained tmms absorb the
inter-stage transposes. Pairs are processed two-per-slot packed into one
complex IFFT (both spectra Hermitian, so ifft2(H_p + i H_q) = corr_p + i corr_q).

Normalization is folded into the spectral domain: fft2((u-mu)/std) =
fft2(u)/std with the DC coefficient forced to 0 (mean-sub only affects DC).
So the raw field FFTs run unscaled, the PSUM->SBUF copy applies 1/std, and a
single strided memset zeroes the DC entries of all 81 spectra.

The per-pair combine Zr = Hr_p - Hi_q, Zi = Hi_p + Hr_q is absorbed into the
first IFFT stage's PSUM accumulation (4 matmuls against +/- copies of the
stage-A weights) instead of DVE ops -- DVE is the pair-loop bottleneck.

Sharding: core c = batch (c//4) x pair-quarter (c%4). Spectra of all 81 fields
live in SBUF as (re|im|im|-re) 512-blocks; per-pair operands are fetched with
register-offset dynamic APs (offsets la*512 from an int32 input table), so one
static graph serves all 8 cores.
"""

import numpy as np
import ml_dtypes

import concourse.bass as bass
import concourse.bacc as bacc
import concourse.tile as tile
import concourse.mybir as mybir
from concourse.bass_utils import run_bass_kernel_spmd

BF16 = mybir.dt.bfloat16
F32 = mybir.dt.float32
I32 = mybir.dt.int32
AF = mybir.ActivationFunctionType
OP = mybir.AluOpType
DVE = mybir.EngineType.DVE

M = 128
NJL = 21          # 20 (j,l) filters + phi as slot 20
NFLD = 81         # 80 alpha fields + normalized phi field as field 80
NPK = 121         # packed pair slots per core (2 pairs each)
NPAIR = 242       # padded pairs per core
QUARTS = [(0, 241), (241, 240), (481, 240), (721, 240)]  # (start, count) of 961
P_TOT = 961

_cache = {}


def _build_nc():
    nc = bacc.Bacc("TRN2", target_bir_lowering=False, debug=False, num_devices=8)

    xin = nc.dram_tensor("xin", [M, M], F32, kind="ExternalInput")
    psid = nc.dram_tensor("psi", [M, NJL * M], BF16, kind="ExternalInput")
    wmatd = nc.dram_tensor("wmat", [M, 16 * M], BF16, kind="ExternalInput")
    wpaird = nc.dram_tensor("wpair", [M, 654], BF16, kind="ExternalInput")
    mgd = nc.dram_tensor("mgath", [65, NPK * 132], BF16, kind="ExternalInput")
    cnsd = nc.dram_tensor("cns", [M, 260], F32, kind="ExternalInput")
    offsd = nc.dram_tensor("offs", [1, NPK * 4], I32, kind="ExternalInput")
    outd = nc.dram_tensor("out", [NPAIR, M, M], BF16, kind="ExternalOutput")

    with tile.TileContext(nc) as tc:
        with (
            tc.tile_pool(name="const", bufs=1) as cp,
            tc.tile_pool(name="work", bufs=4) as wp,
            tc.tile_pool(name="pairw", bufs=5) as pw,
            tc.tile_pool(name="ps256", bufs=2, space="PSUM") as pp,
            tc.tile_pool(name="ps512", bufs=3, space="PSUM") as pv,
            tc.tile_pool(name="psB", bufs=3, space="PSUM") as pq,
        ):
            # ---- constants into SBUF ----
            wsb = cp.tile([M, 16 * M], BF16)
            nc.sync.dma_start(wsb, wmatd.ap())
            wpsb = cp.tile([M, 654], BF16)
            nc.sync.dma_start(wpsb, wpaird.ap())
            psisb = cp.tile([M, NJL * M], BF16)
            nc.sync.dma_start(psisb, psid.ap())
            MG = cp.tile([65, NPK * 132], BF16)
            nc.sync.dma_start(MG, mgd.ap())
            cns = cp.tile([M, 260], F32)
            nc.sync.dma_start(cns, cnsd.ap())
            offsb = cp.tile([1, NPK * 4], I32)
            nc.sync.dma_start(offsb, offsd.ap())
            xf = cp.tile([M, M], F32)
            nc.sync.dma_start(xf, xin.ap())

            # zero-fill the never-written regions of out: mid rows 33..95 and
            # dead cols 33..94 of the live rows. Few big DMAs, 4 queue-chunks.
            zsb = cp.tile([M, M], BF16)
            nc.vector.memzero(zsb)
            for j0 in range(0, NPAIR, 61):
                n = min(61, NPAIR - j0)
                # rows 33..95 full width
                dst = bass.AP(outd, j0 * M * M + 33 * M,
                              [[M, 63], [M * M, n], [1, M]])
                src = zsb[0:63, 0:M].rearrange("p (one x) -> p one x", one=1)
                nc.sync.dma_start(dst, src.to_broadcast((63, n, M)))
                # rows 0..32, cols 33..94
                dst = bass.AP(outd, j0 * M * M + 33,
                              [[M, 33], [M * M, n], [1, 62]])
                src = zsb[0:33, 0:62].rearrange("p (one x) -> p one x", one=1)
                nc.sync.dma_start(dst, src.to_broadcast((33, n, 62)))
                # rows 96..127, cols 33..94
                dst = bass.AP(outd, j0 * M * M + 96 * M + 33,
                              [[M, 32], [M * M, n], [1, 62]])
                src = zsb[0:32, 0:62].rearrange("p (one x) -> p one x", one=1)
                nc.sync.dma_start(dst, src.to_broadcast((32, n, 62)))

            # big bf16 staging buffer for all live-window outputs
            OGB = cp.tile([65, NPK * 132], BF16)

            def WB(i, n=1):
                return wsb[:, i * M:(i + n) * M]

            ones_col = cns[:, 0:1]          # [128,1] f32 of 1.0
            ones_row = cns[0:1, 130:258]    # [1,128] f32 of 1.0
            ident = cns[:, 2:130]           # [128,128] f32 identity

            # persistent SBUF state
            SPBIG = cp.tile([M, NFLD * 512], BF16)  # (re|im|im|-re) per field
            FLDS = cp.tile([M, NFLD * M], BF16)     # relu'd fields
            HX = cp.tile([M, 256], BF16)            # hatx (re|im)
            SUMS = cp.tile([M, NFLD], F32)
            SQS = cp.tile([M, NFLD], F32)
            AB = cp.tile([M, NFLD], F32)            # per-field 1/std bcast

            # ---- hatx = fft2(x) ----
            xb = wp.tile([M, M], BF16, tag="xb")
            nc.scalar.activation(xb, xf, AF.Copy)
            psF = pp.tile([M, 256], F32, tag="ps256")
            nc.tensor.matmul(psF, xb, WB(0, 2), start=True, stop=True)
            fsb = wp.tile([M, 256], BF16, tag="fsb")
            nc.scalar.activation(fsb, psF, AF.Copy)
            psH = pp.tile([M, 256], F32, tag="ps256")
            nc.tensor.matmul(psH, fsb[:, 0:M], WB(0, 2), start=True, stop=False)
            nc.tensor.matmul(psH, fsb[:, M:256], WB(2, 2), start=False, stop=True)
            nc.scalar.activation(HX, psH, AF.Copy)

            # ---- z_jl = ifft2(hatx * psi_jl); fields + row-sums ----
            HX3 = HX[:, 0:256].rearrange("p (two c) -> p two c", two=2)
            for jl in range(NJL):
                pj = psisb[:, jl * M:(jl + 1) * M]
                pj3 = pj.rearrange("p (one c) -> p one c", one=1)
                ab2 = wp.tile([M, 256], BF16, tag="ab2")
                nc.vector.tensor_tensor(
                    ab2.rearrange("p (two c) -> p two c", two=2),
                    HX3, pj3.to_broadcast((M, 2, M)), OP.mult)
                psT = pp.tile([M, 256], F32, tag="ps256")
                nc.tensor.matmul(psT, ab2[:, 0:M], WB(4, 2), start=True, stop=False)
                nc.tensor.matmul(psT, ab2[:, M:256], WB(6, 2), start=False, stop=True)
                tsb = wp.tile([M, 256], BF16, tag="tsb")
                if jl % 2 == 0:
                    nc.scalar.activation(tsb, psT, AF.Copy)
                else:
                    nc.vector.tensor_scalar(tsb, psT, 1.0, None, OP.mult)
                psZ = pp.tile([M, 256], F32, tag="ps256")
                nc.tensor.matmul(psZ, tsb[:, 0:M], WB(4, 2), start=True, stop=False)
                nc.tensor.matmul(psZ, tsb[:, M:256], WB(6, 2), start=False, stop=True)
                if jl < 20:
                    # fields 4jl+a: relu(+zr), relu(-zi), relu(-zr), relu(+zi)
                    # a=0,1 on ACT (relu + fused row-sum), a=2,3 on DVE
                    for a, (half, sc) in enumerate(
                        [(0, 1.0), (1, -1.0), (0, -1.0), (1, 1.0)]
                    ):
                        k = 4 * jl + a
                        fld = FLDS[:, k * M:(k + 1) * M]
                        src = psZ[:, half * M:(half + 1) * M]
                        if a < 2:
                            nc.scalar.activation(
                                fld, src, AF.Relu,
                                scale=sc, accum_out=SUMS[:, k:k + 1],
                            )
                        else:
                            nc.vector.tensor_scalar(
                                fld, src, sc, 0.0, OP.mult, OP.max)
                            nc.vector.tensor_reduce(
                                SUMS[:, k:k + 1], fld, mybir.AxisListType.X, OP.add)
                    # squared sums: a=0,1 on ACT Square, a=2,3 on DVE
                    for a in range(4):
                        k = 4 * jl + a
                        fld = FLDS[:, k * M:(k + 1) * M]
                        dump = wp.tile([M, M], BF16, tag="dump")
                        if a < 2:
                            nc.scalar.activation(
                                dump, fld, AF.Square, accum_out=SQS[:, k:k + 1])
                        else:
                            nc.vector.tensor_tensor(dump, fld, fld, OP.mult)
                            nc.vector.tensor_reduce(
                                SQS[:, k:k + 1], dump, mybir.AxisListType.X, OP.add)
                else:
                    k = 80
                    fld = FLDS[:, k * M:(k + 1) * M]
                    nc.scalar.activation(
                        fld, psZ[:, 0:M], AF.Copy,
                        accum_out=SUMS[:, k:k + 1],
                    )
                    dump = wp.tile([M, M], BF16, tag="dump")
                    nc.scalar.activation(
                        dump, fld, AF.Square, accum_out=SQS[:, k:k + 1])

            # ---- stats: total mean/var -> alpha = 1/std, bcast to [128, 81] ----
            psS = pp.tile([M, 256], F32, tag="ps256")
            nc.tensor.matmul(psS[0:NFLD, 0:1], SUMS, ones_col, start=True, stop=True)
            nc.tensor.matmul(psS[0:NFLD, 1:2], SQS, ones_col, start=True, stop=True)
            st = wp.tile([NFLD, 2], F32, tag="st1")
            nc.scalar.activation(st, psS[0:NFLD, 0:2], AF.Copy, scale=1.0 / 16384.0)
            mu = st[:, 0:1]
            e2 = st[:, 1:2]
            mu2 = wp.tile([NFLD, 1], F32, tag="mu2")
            nc.vector.tensor_tensor(mu2, mu, mu, OP.mult)
            var = wp.tile([NFLD, 1], F32, tag="var")
            nc.vector.tensor_tensor(var, e2, mu2, OP.subtract)
            alph = wp.tile([NFLD, 1], F32, tag="alph")
            sdev = wp.tile([NFLD, 1], F32, tag="sdev")
            nc.scalar.activation(sdev, var, AF.Sqrt)
            nc.vector.reciprocal(alph, sdev)
            psS2 = pp.tile([M, 256], F32, tag="ps256")
            nc.tensor.transpose(psS2[0:1, 0:NFLD], alph, ident[0:NFLD, 0:NFLD])
            arow = wp.tile([1, NFLD], F32, tag="arow")
            nc.scalar.activation(arow, psS2[0:1, 0:NFLD], AF.Copy)
            psAB = pp.tile([M, 256], F32, tag="ps256")
            nc.tensor.matmul(psAB[:, 0:NFLD], ones_row, arow, start=True, stop=True)
            nc.scalar.activation(AB, psAB[:, 0:NFLD], AF.Copy)

            # ---- forward fft per field (unnormalized) -> SPBIG * (1/std) ----
            for k in range(NFLD):
                fld = FLDS[:, k * M:(k + 1) * M]
                psF2 = pp.tile([M, 256], F32, tag="ps256")
                nc.tensor.matmul(psF2, fld, WB(0, 2), start=True, stop=True)
                f2 = wp.tile([M, 256], BF16, tag="fsb")
                if k % 2 == 0:
                    nc.scalar.activation(f2, psF2, AF.Copy)
                else:
                    nc.vector.tensor_scalar(f2, psF2, 1.0, None, OP.mult)
                psH5 = pv.tile([M, 512], F32, tag="ps512")
                nc.tensor.matmul(psH5, f2[:, 0:M], WB(8, 4), start=True, stop=False)
                nc.tensor.matmul(psH5, f2[:, M:256], WB(12, 4), start=False, stop=True)
                dst = SPBIG[:, k * 512:(k + 1) * 512]
                if k % 2 == 0:
                    nc.vector.tensor_scalar(dst, psH5, AB[:, k:k + 1], None, OP.mult)
                else:
                    nc.scalar.activation(dst, psH5, AF.Copy, scale=AB[:, k:k + 1])
            # mean-sub == zero the DC entry of every spectrum (all 4 sub-blocks)
            spz = SPBIG[:].rearrange("p (k c) -> p k c", c=512)
            for off in range(0, 512, 128):
                nc.vector.memset(spz[0:1, :, off:off + 1], 0.0)

            # ---- packed pair loop: slot s covers pairs (2s, 2s+1) ----
            vals = None
            psB = None
            for s in range(NPK):
                if s % 8 == 0:
                    nv = min(8, NPK - s) * 4
                    _, vals = nc.values_load_multi_w_load_instructions(
                        offsb[0:1, 4 * s:4 * s + nv],
                        engines=[DVE],
                        min_val=0, max_val=80 * 512,
                        skip_runtime_bounds_check=True,
                    )
                o1p, o2p, o1q, o2q = vals[4 * (s % 8):4 * (s % 8) + 4]

                # products: (t1|t2|t3|-t4) per pair; lhs is the (re|im|im|-re)
                # block of la1, rhs the (re|im) half of la2 repeated twice
                prod = pw.tile([M, 1024], BF16, tag="prod")
                rhs_p = SPBIG[:, bass.ds(o2p, 256)].rearrange(
                    "p (one c) -> p one c", one=1).to_broadcast((M, 2, 256))
                rhs_q = SPBIG[:, bass.ds(o2q, 256)].rearrange(
                    "p (one c) -> p one c", one=1).to_broadcast((M, 2, 256))
                lhs_p = SPBIG[:, bass.ds(o1p, 512)].rearrange(
                    "p (two c) -> p two c", two=2)
                lhs_q = SPBIG[:, bass.ds(o1q, 512)].rearrange(
                    "p (two c) -> p two c", two=2)
                nc.vector.tensor_tensor(
                    prod[:, 0:512].rearrange("p (two c) -> p two c", two=2),
                    lhs_p, rhs_p, OP.mult)
                nc.vector.tensor_tensor(
                    prod[:, 512:1024].rearrange("p (two c) -> p two c", two=2),
                    lhs_q, rhs_q, OP.mult)

                # X = (Hr_p|Hi_p|Hr_q|Hi_q): pairwise sums of adjacent blocks
                X = pw.tile([M, 512], BF16, tag="X")
                pr3 = prod[:].rearrange("p (n two c) -> p n two c", two=2, c=M)
                xeng = nc.gpsimd if s % 4 != 1 else nc.vector
                xeng.tensor_tensor(
                    X[:].rearrange("p (n c) -> p n c", c=M),
                    pr3[:, :, 0, :], pr3[:, :, 1, :], OP.add)

                # packed ifft stage A with the Zr/Zi +/- combine folded into
                # PSUM accumulation: Zr = X0 - X3 (wA0), Zi = X1 + X2 (wA1)
                psA = pv.tile([M, 130], F32, tag="ps512",
                              padded_shape=[M, 512])
                nc.tensor.matmul(psA, X[:, 0:M], wpsb[:, 0:130], start=True, stop=False)
                nc.tensor.matmul(psA, X[:, M:256], wpsb[:, 130:260], start=False, stop=False)
                nc.tensor.matmul(psA, X[:, 256:384], wpsb[:, 130:260], start=False, stop=False)
                nc.tensor.matmul(psA, X[:, 384:512], wpsb[:, 524:654], start=False, stop=True)
                t1sb = pw.tile([M, 130], BF16, tag="t1sb")
                nc.scalar.activation(t1sb, psA, AF.Copy)

                # stage B batched 3 slots per PSUM bank; one masked copy-out
                g, r = divmod(s, 3)
                if r == 0:
                    psB = pq.tile([65, 396], F32, tag="psB")
                pBs = psB[:, r * 132:(r + 1) * 132]
                nc.tensor.matmul(pBs, t1sb[:, 0:65], wpsb[:, 260:392], start=True, stop=False)
                nc.tensor.matmul(pBs, t1sb[:, 65:130], wpsb[:, 392:524], start=False, stop=True)
                if r == 2 or s == NPK - 1:
                    s0 = 3 * g
                    w = (r + 1) * 132
                    nc.vector.tensor_tensor(
                        OGB[:, 132 * s0:132 * s0 + w], psB[:, 0:w],
                        MG[:, 132 * s0:132 * s0 + w], OP.mult)

            # 4 big output DMAs: (rows 0..32 | 96..127) x (cols 0..32 | 95..127)
            ogp = OGB[:].rearrange("p (pair c v) -> p pair c v", c=2, v=33)
            for (r0, rn, po) in [(0, 33, 0), (33, 32, 96 * M)]:
                for (cj, co) in [(0, 0), (1, 95)]:
                    dst = bass.AP(outd, po + co,
                                  [[M, rn], [M * M, NPAIR], [1, 33]])
                    nc.sync.dma_start(dst, ogp[r0:r0 + rn, :, cj])

    nc.compile()
    return nc


def _host_tables(la1, la2, shifted):
    """Per-core (offs [1,NPK*4] int32, mask-gather source indices [NPAIR])."""
    la1 = np.concatenate([la1.astype(np.int64), [80]])     # phi pair appended
    la2 = np.concatenate([la2.astype(np.int64), [80]])
    sh = np.concatenate([shifted.astype(np.int64), [5]])
    offs, mgi = [], []
    for (st, cnt) in QUARTS:
        t = np.zeros((NPAIR, 2), np.int64)
        mi = np.zeros(NPAIR, np.int64)
        idx = np.arange(st, st + cnt)
        t[:cnt, 0] = la1[idx] * 512
        t[:cnt, 1] = la2[idx] * 512
        mi[:cnt] = sh[idx]
        o = np.zeros((NPK, 4), np.int32)
        o[:, 0] = t[0::2, 0]
        o[:, 1] = t[0::2, 1]
        o[:, 2] = t[1::2, 0]
        o[:, 3] = t[1::2, 1]
        offs.append(o.reshape(1, -1))
        mgi.append(mi)
    return offs, mgi


def _host_consts():
    k = np.arange(M)
    W = np.exp(-2j * np.pi * np.outer(k, k) / M)
    Wr = W.real.astype(np.float32)
    Wi = W.imag.astype(np.float32)
    V = np.conj(W) / M
    Ar = V.real.astype(np.float32)
    Ai = V.imag.astype(np.float32)
    Pr = Ar / 16384.0
    Pi = Ai / 16384.0
    blocks = [Wr, Wi, -Wi, Wr,            # 0-3: fwd stage1 + hatx stage2
              Ar, Ai, -Ai, Ar,            # 4-7: inverse stages
              Wr, Wi, Wi, -Wr,            # 8-11: fwd stage2 rhs1 (512)
              -Wi, Wr, Wr, Wi]            # 12-15: fwd stage2 rhs2 (512)
    wmat = np.concatenate(blocks, axis=1).astype(ml_dtypes.bfloat16)
    sa = np.r_[0:33, 96:128]
    sb = np.r_[0:33, 95:128]
    wpair = np.concatenate(
        [Ar[:, sa], Ai[:, sa], -Ai[:, sa], Ar[:, sa],
         Pr[:, sb], Pi[:, sb], -Pi[:, sb], Pr[:, sb],
         -Ar[:, sa], -Ai[:, sa]], axis=1
    ).astype(ml_dtypes.bfloat16)
    cns = np.zeros((M, 260), np.float32)
    cns[:, 0] = 1.0
    cns[:, 2:130] = np.eye(M, dtype=np.float32)
    cns[:, 130:258] = 1.0
    return wmat, wpair, cns


def _prepare(inputs):
    x = np.asarray(inputs["x"], np.float32)
    hatpsi = np.asarray(inputs["hatpsi"], np.float32)
    hatphi = np.asarray(inputs["hatphi"], np.float32)
    masks = np.asarray(inputs["masks_shift"], np.float32)
    la1 = np.asarray(inputs["la1"])
    la2 = np.asarray(inputs["la2"])
    shifted = np.asarray(inputs["shifted"])

    wmat, wpair, cns = _host_consts()
    psi = np.concatenate(
        [hatpsi.transpose(2, 0, 1, 3).reshape(M, 20 * M), hatphi], axis=1
    ).astype(ml_dtypes.bfloat16)
    offs, mgi = _host_tables(la1, la2, shifted)
    masks_bf = masks.astype(ml_dtypes.bfloat16)
    sa = np.r_[0:33, 96:128]
    sb = np.r_[0:33, 95:128]

    in_maps = []
    for c in range(8):
        b, q = c // 4, c % 4
        mg = masks_bf[mgi[q]]                       # [NPAIR,128,128]
        mg = mg[:, sa][:, :, sb]                    # [NPAIR,65,66]
        mg = np.ascontiguousarray(mg.transpose(1, 0, 2)).reshape(65, NPAIR * 66)
        in_maps.append({
            "xin": np.ascontiguousarray(x[b, 0]),
            "psi": psi,
            "wmat": wmat,
            "wpair": wpair,
            "mgath": mg,
            "cns": cns,
            "offs": offs[q],
        })
    return in_maps


def _assemble(results):
    out = np.empty((2, P_TOT, M * M), np.float32)
    for c in range(8):
        b, q = c // 4, c % 4
        s, cnt = QUARTS[q]
        r = results[c]["out"].reshape(NPAIR, M * M)
        out[b, s:s + cnt] = r[:cnt].astype(np.float32)
    return out.reshape(2, -1)


def kernel(**inputs):
    if "nc" not in _cache:
        _cache["nc"] = _build_nc()
    nc = _cache["nc"]
    in_maps = _prepare(inputs)
    res = run_bass_kernel_spmd(nc, in_maps, core_ids=list(range(8)))
    return _assemble(res.results)


def kernel_traced(tmpdir=None, **inputs):
    """Like kernel() but with neuron-profile tracing; returns (out, results)."""
    if "nc" not in _cache:
        _cache["nc"] = _build_nc()
    nc = _cache["nc"]
    in_maps = _prepare(inputs)
    res = run_bass_kernel_spmd(
        nc, in_maps, core_ids=list(range(8)), trace=True, tmpdir=tmpdir
    )
    return _assemble(res.results), res


# revision 9
# speedup vs baseline: 1.0029x; 1.0029x over previous
"""ALPHA scattering-covariance kernel for 8 Trainium2 NeuronCores.

Math (per batch element b, nc=1, M=N=128, J=5 L=4 A=4, P=960 pairs + 1 phi):
  hatx = fft2(x)
  z_jl = ifft2(hatx * hatpsi[j,l])           (20 complex fields)
  field k=4*jl+a: u = relu(phase_a(z_jl)),   phase in {Re, -Im, -Re, +Im}
  phi field (k=80): u = Re(ifft2(hatx*hatphi))   (no relu)
  n_k = (u - mean)/std ;  hat_n_k = fft2(n_k)
  pair p: corr = Re(ifft2(hat_n[la1] * conj(hat_n[la2]))) / (M*N) * masks[shifted]
  out[b] = concat(pairs 0..959, phi autocorr)

2D FFTs run as transpose-matmuls on the TensorEngine: tmm(P, Q) := P^T @ Q with
P stationary; fft2(n) = W n W (W symmetric), two chained tmms absorb the
inter-stage transposes. Pairs are processed two-per-slot packed into one
complex IFFT (both spectra Hermitian, so ifft2(H_p + i H_q) = corr_p + i corr_q).

Normalization is folded into the spectral domain: fft2((u-mu)/std) =
fft2(u)/std with the DC coefficient forced to 0 (mean-sub only affects DC).
So the raw field FFTs run unscaled, the PSUM->SBUF copy applies 1/std, and a
single strided memset zeroes the DC entries of all 81 spectra.

The per-pair combine Zr = Hr_p - Hi_q, Zi = Hi_p + Hr_q is absorbed into the
first IFFT stage's PSUM accumulation (4 matmuls against +/- copies of the
stage-A weights) instead of DVE ops -- DVE is the pair-loop bottleneck.

Sharding: core c = batch (c//4) x pair-quarter (c%4). Spectra of all 81 fields
live in SBUF as (re|im|im|-re) 512-blocks; per-pair operands are fetched with
register-offset dynamic APs (offsets la*512 from an int32 input table), so one
static graph serves all 8 cores.
"""

import numpy as np
import ml_dtypes

import concourse.bass as bass
import concourse.bacc as bacc
import concourse.tile as tile
import concourse.mybir as mybir
from concourse.bass_utils import run_bass_kernel_spmd

BF16 = mybir.dt.bfloat16
F32 = mybir.dt.float32
I32 = mybir.dt.int32
AF = mybir.ActivationFunctionType
OP = mybir.AluOpType
DVE = mybir.EngineType.DVE

M = 128
NJL = 21          # 20 (j,l) filters + phi as slot 20
NFLD = 81         # 80 alpha fields + normalized phi field as field 80
NPK = 121         # packed pair slots per core (2 pairs each)
NPAIR = 242       # padded pairs per core
QUARTS = [(0, 241), (241, 240), (481, 240), (721, 240)]  # (start, count) of 961
P_TOT = 961

_cache = {}


def _build_nc():
    nc = bacc.Bacc("TRN2", target_bir_lowering=False, debug=False, num_devices=8)

    xin = nc.dram_tensor("xin", [M, M], F32, kind="ExternalInput")
    psid = nc.dram_tensor("psi", [M, NJL * M], BF16, kind="ExternalInput")
    wmatd = nc.dram_tensor("wmat", [M, 16 * M], BF16, kind="ExternalInput")
    wpaird = nc.dram_tensor("wpair", [M, 654], BF16, kind="ExternalInput")
    mgd = nc.dram_tensor("mgath", [65, NPK * 132], BF16, kind="ExternalInput")
    cnsd = nc.dram_tensor("cns", [M, 260], F32, kind="ExternalInput")
    offsd = nc.dram_tensor("offs", [1, NPK * 4], I32, kind="ExternalInput")
    outd = nc.dram_tensor("out", [NPAIR, M, M], BF16, kind="ExternalOutput")

    with tile.TileContext(nc) as tc:
        with (
            tc.tile_pool(name="const", bufs=1) as cp,
            tc.tile_pool(name="work", bufs=3) as wp,
            tc.tile_pool(name="pairw", bufs=4) as pw,
            tc.tile_pool(name="ps256", bufs=2, space="PSUM") as pp,
            tc.tile_pool(name="ps512", bufs=3, space="PSUM") as pv,
            tc.tile_pool(name="psB", bufs=3, space="PSUM") as pq,
        ):
            # ---- constants into SBUF ----
            wsb = cp.tile([M, 16 * M], BF16)
            nc.sync.dma_start(wsb, wmatd.ap())
            wpsb = cp.tile([M, 654], BF16)
            nc.sync.dma_start(wpsb, wpaird.ap())
            psisb = cp.tile([M, NJL * M], BF16)
            nc.sync.dma_start(psisb, psid.ap())
            MG = cp.tile([65, NPK * 132], BF16)
            nc.sync.dma_start(MG, mgd.ap())
            cns = cp.tile([M, 260], F32)
            nc.sync.dma_start(cns, cnsd.ap())
            offsb = cp.tile([1, NPK * 4], I32)
            nc.sync.dma_start(offsb, offsd.ap())
            xf = cp.tile([M, M], F32)
            nc.sync.dma_start(xf, xin.ap())

            # zero-fill the never-written regions of out: mid rows 33..95 and
            # dead cols 33..94 of the live rows. Few big DMAs, 4 queue-chunks.
            zsb = cp.tile([M, M], BF16)
            nc.vector.memzero(zsb)
            for j0 in range(0, NPAIR, 61):
                n = min(61, NPAIR - j0)
                # rows 33..95 full width
                dst = bass.AP(outd, j0 * M * M + 33 * M,
                              [[M, 63], [M * M, n], [1, M]])
                src = zsb[0:63, 0:M].rearrange("p (one x) -> p one x", one=1)
                nc.sync.dma_start(dst, src.to_broadcast((63, n, M)))
                # rows 0..32, cols 33..94
                dst = bass.AP(outd, j0 * M * M + 33,
                              [[M, 33], [M * M, n], [1, 62]])
                src = zsb[0:33, 0:62].rearrange("p (one x) -> p one x", one=1)
                nc.sync.dma_start(dst, src.to_broadcast((33, n, 62)))
                # rows 96..127, cols 33..94
                dst = bass.AP(outd, j0 * M * M + 96 * M + 33,
                              [[M, 32], [M * M, n], [1, 62]])
                src = zsb[0:32, 0:62].rearrange("p (one x) -> p one x", one=1)
                nc.sync.dma_start(dst, src.to_broadcast((32, n, 62)))

            # big bf16 staging buffer for all live-window outputs
            OGB = cp.tile([65, NPK * 132], BF16)

            def WB(i, n=1):
                return wsb[:, i * M:(i + n) * M]

            ones_col = cns[:, 0:1]          # [128,1] f32 of 1.0
            ones_row = cns[0:1, 130:258]    # [1,128] f32 of 1.0
            ident = cns[:, 2:130]           # [128,128] f32 identity

            # persistent SBUF state
            SPBIG = cp.tile([M, NFLD * 512], BF16)  # (re|im|im|-re) per field
            FLDS = cp.tile([M, NFLD * M], BF16)     # relu'd fields
            HX = cp.tile([M, 256], BF16)            # hatx (re|im)
            SUMS = cp.tile([M, NFLD], F32)
            SQS = cp.tile([M, NFLD], F32)
            AB = cp.tile([M, NFLD], F32)            # per-field 1/std bcast

            # ---- hatx = fft2(x) ----
            xb = wp.tile([M, M], BF16, tag="xb")
            nc.scalar.activation(xb, xf, AF.Copy)
            psF = pp.tile([M, 256], F32, tag="ps256")
            nc.tensor.matmul(psF, xb, WB(0, 2), start=True, stop=True)
            fsb = wp.tile([M, 256], BF16, tag="fsb")
            nc.scalar.activation(fsb, psF, AF.Copy)
            psH = pp.tile([M, 256], F32, tag="ps256")
            nc.tensor.matmul(psH, fsb[:, 0:M], WB(0, 2), start=True, stop=False)
            nc.tensor.matmul(psH, fsb[:, M:256], WB(2, 2), start=False, stop=True)
            nc.scalar.activation(HX, psH, AF.Copy)

            # ---- z_jl = ifft2(hatx * psi_jl); fields + row-sums ----
            HX3 = HX[:, 0:256].rearrange("p (two c) -> p two c", two=2)
            for jl in range(NJL):
                pj = psisb[:, jl * M:(jl + 1) * M]
                pj3 = pj.rearrange("p (one c) -> p one c", one=1)
                ab2 = wp.tile([M, 256], BF16, tag="ab2")
                nc.vector.tensor_tensor(
                    ab2.rearrange("p (two c) -> p two c", two=2),
                    HX3, pj3.to_broadcast((M, 2, M)), OP.mult)
                psT = pp.tile([M, 256], F32, tag="ps256")
                nc.tensor.matmul(psT, ab2[:, 0:M], WB(4, 2), start=True, stop=False)
                nc.tensor.matmul(psT, ab2[:, M:256], WB(6, 2), start=False, stop=True)
                tsb = wp.tile([M, 256], BF16, tag="tsb")
                if jl % 2 == 0:
                    nc.scalar.activation(tsb, psT, AF.Copy)
                else:
                    nc.vector.tensor_scalar(tsb, psT, 1.0, None, OP.mult)
                psZ = pp.tile([M, 256], F32, tag="ps256")
                nc.tensor.matmul(psZ, tsb[:, 0:M], WB(4, 2), start=True, stop=False)
                nc.tensor.matmul(psZ, tsb[:, M:256], WB(6, 2), start=False, stop=True)
                if jl < 20:
                    # fields 4jl+a: relu(+zr), relu(-zi), relu(-zr), relu(+zi)
                    # a=0,1 on ACT (relu + fused row-sum), a=2,3 on DVE
                    for a, (half, sc) in enumerate(
                        [(0, 1.0), (1, -1.0), (0, -1.0), (1, 1.0)]
                    ):
                        k = 4 * jl + a
                        fld = FLDS[:, k * M:(k + 1) * M]
                        src = psZ[:, half * M:(half + 1) * M]
                        if a < 2:
                            nc.scalar.activation(
                                fld, src, AF.Relu,
                                scale=sc, accum_out=SUMS[:, k:k + 1],
                            )
                        else:
                            nc.vector.tensor_scalar(
                                fld, src, sc, 0.0, OP.mult, OP.max)
                            nc.vector.tensor_reduce(
                                SUMS[:, k:k + 1], fld, mybir.AxisListType.X, OP.add)
                    # squared sums: a=0,1 on ACT Square, a=2,3 on DVE
                    for a in range(4):
                        k = 4 * jl + a
                        fld = FLDS[:, k * M:(k + 1) * M]
                        dump = wp.tile([M, M], BF16, tag="dump")
                        if a < 2:
                            nc.scalar.activation(
                                dump, fld, AF.Square, accum_out=SQS[:, k:k + 1])
                        else:
                            nc.vector.tensor_tensor(dump, fld, fld, OP.mult)
                            nc.vector.tensor_reduce(
                                SQS[:, k:k + 1], dump, mybir.AxisListType.X, OP.add)
                else:
                    k = 80
                    fld = FLDS[:, k * M:(k + 1) * M]
                    nc.scalar.activation(
                        fld, psZ[:, 0:M], AF.Copy,
                        accum_out=SUMS[:, k:k + 1],
                    )
                    dump = wp.tile([M, M], BF16, tag="dump")
                    nc.scalar.activation(
                        dump, fld, AF.Square, accum_out=SQS[:, k:k + 1])

            # ---- stats: total mean/var -> alpha = 1/std, bcast to [128, 81] ----
            psS = pp.tile([M, 256], F32, tag="ps256")
            nc.tensor.matmul(psS[0:NFLD, 0:1], SUMS, ones_col, start=True, stop=True)
            nc.tensor.matmul(psS[0:NFLD, 1:2], SQS, ones_col, start=True, stop=True)
            st = wp.tile([NFLD, 2], F32, tag="st1")
            nc.scalar.activation(st, psS[0:NFLD, 0:2], AF.Copy, scale=1.0 / 16384.0)
            mu = st[:, 0:1]
            e2 = st[:, 1:2]
            mu2 = wp.tile([NFLD, 1], F32, tag="mu2")
            nc.vector.tensor_tensor(mu2, mu, mu, OP.mult)
            var = wp.tile([NFLD, 1], F32, tag="var")
            nc.vector.tensor_tensor(var, e2, mu2, OP.subtract)
            alph = wp.tile([NFLD, 1], F32, tag="alph")
            sdev = wp.tile([NFLD, 1], F32, tag="sdev")
            nc.scalar.activation(sdev, var, AF.Sqrt)
            nc.vector.reciprocal(alph, sdev)
            psS2 = pp.tile([M, 256], F32, tag="ps256")
            nc.tensor.transpose(psS2[0:1, 0:NFLD], alph, ident[0:NFLD, 0:NFLD])
            arow = wp.tile([1, NFLD], F32, tag="arow")
            nc.scalar.activation(arow, psS2[0:1, 0:NFLD], AF.Copy)
            psAB = pp.tile([M, 256], F32, tag="ps256")
            nc.tensor.matmul(psAB[:, 0:NFLD], ones_row, arow, start=True, stop=True)
            nc.scalar.activation(AB, psAB[:, 0:NFLD], AF.Copy)

            # ---- forward fft per field (unnormalized) -> SPBIG * (1/std) ----
            for k in range(NFLD):
                fld = FLDS[:, k * M:(k + 1) * M]
                psF2 = pp.tile([M, 256], F32, tag="ps256")
                nc.tensor.matmul(psF2, fld, WB(0, 2), start=True, stop=True)
                f2 = wp.tile([M, 256], BF16, tag="fsb")
                if k % 2 == 0:
                    nc.scalar.activation(f2, psF2, AF.Copy)
                else:
                    nc.vector.tensor_scalar(f2, psF2, 1.0, None, OP.mult)
                psH5 = pv.tile([M, 512], F32, tag="ps512")
                nc.tensor.matmul(psH5, f2[:, 0:M], WB(8, 4), start=True, stop=False)
                nc.tensor.matmul(psH5, f2[:, M:256], WB(12, 4), start=False, stop=True)
                dst = SPBIG[:, k * 512:(k + 1) * 512]
                if k % 2 == 0:
                    nc.vector.tensor_scalar(dst, psH5, AB[:, k:k + 1], None, OP.mult)
                else:
                    nc.scalar.activation(dst, psH5, AF.Copy, scale=AB[:, k:k + 1])
            # mean-sub == zero the DC entry of every spectrum (all 4 sub-blocks)
            spz = SPBIG[:].rearrange("p (k c) -> p k c", c=512)
            for off in range(0, 512, 128):
                nc.vector.memset(spz[0:1, :, off:off + 1], 0.0)

            # ---- packed pair loop: slot s covers pairs (2s, 2s+1) ----
            vals = None
            psB = None
            for s in range(NPK):
                if s % 8 == 0:
                    nv = min(8, NPK - s) * 4
                    _, vals = nc.values_load_multi_w_load_instructions(
                        offsb[0:1, 4 * s:4 * s + nv],
                        engines=[DVE],
                        min_val=0, max_val=80 * 512,
                        skip_runtime_bounds_check=True,
                    )
                o1p, o2p, o1q, o2q = vals[4 * (s % 8):4 * (s % 8) + 4]

                # products: (t1|t2|t3|-t4) per pair; lhs is the (re|im|im|-re)
                # block of la1, rhs the (re|im) half of la2 repeated twice
                prod = pw.tile([M, 1024], BF16, tag="prod")
                rhs_p = SPBIG[:, bass.ds(o2p, 256)].rearrange(
                    "p (one c) -> p one c", one=1).to_broadcast((M, 2, 256))
                rhs_q = SPBIG[:, bass.ds(o2q, 256)].rearrange(
                    "p (one c) -> p one c", one=1).to_broadcast((M, 2, 256))
                lhs_p = SPBIG[:, bass.ds(o1p, 512)].rearrange(
                    "p (two c) -> p two c", two=2)
                lhs_q = SPBIG[:, bass.ds(o1q, 512)].rearrange(
                    "p (two c) -> p two c", two=2)
                nc.vector.tensor_tensor(
                    prod[:, 0:512].rearrange("p (two c) -> p two c", two=2),
                    lhs_p, rhs_p, OP.mult)
                nc.vector.tensor_tensor(
                    prod[:, 512:1024].rearrange("p (two c) -> p two c", two=2),
                    lhs_q, rhs_q, OP.mult)

                # X = (Hr_p|Hi_p|Hr_q|Hi_q): pairwise sums of adjacent blocks
                X = pw.tile([M, 512], BF16, tag="X")
                pr3 = prod[:].rearrange("p (n two c) -> p n two c", two=2, c=M)
                xeng = nc.gpsimd if s % 4 != 1 else nc.vector
                xeng.tensor_tensor(
                    X[:].rearrange("p (n c) -> p n c", c=M),
                    pr3[:, :, 0, :], pr3[:, :, 1, :], OP.add)

                # packed ifft stage A with the Zr/Zi +/- combine folded into
                # PSUM accumulation: Zr = X0 - X3 (wA0), Zi = X1 + X2 (wA1)
                psA = pv.tile([M, 130], F32, tag="ps512",
                              padded_shape=[M, 512])
                nc.tensor.matmul(psA, X[:, 0:M], wpsb[:, 0:130], start=True, stop=False)
                nc.tensor.matmul(psA, X[:, M:256], wpsb[:, 130:260], start=False, stop=False)
                nc.tensor.matmul(psA, X[:, 256:384], wpsb[:, 130:260], start=False, stop=False)
                nc.tensor.matmul(psA, X[:, 384:512], wpsb[:, 524:654], start=False, stop=True)
                t1sb = pw.tile([M, 130], BF16, tag="t1sb")
                nc.scalar.activation(t1sb, psA, AF.Copy)

                # stage B batched 3 slots per PSUM bank; one masked copy-out
                g, r = divmod(s, 3)
                if r == 0:
                    psB = pq.tile([65, 396], F32, tag="psB")
                pBs = psB[:, r * 132:(r + 1) * 132]
                nc.tensor.matmul(pBs, t1sb[:, 0:65], wpsb[:, 260:392], start=True, stop=False)
                nc.tensor.matmul(pBs, t1sb[:, 65:130], wpsb[:, 392:524], start=False, stop=True)
                if r == 2 or s == NPK - 1:
                    s0 = 3 * g
                    w = (r + 1) * 132
                    nc.vector.tensor_tensor(
                        OGB[:, 132 * s0:132 * s0 + w], psB[:, 0:w],
                        MG[:, 132 * s0:132 * s0 + w], OP.mult)

            # 4 big output DMAs: (rows 0..32 | 96..127) x (cols 0..32 | 95..127)
            ogp = OGB[:].rearrange("p (pair c v) -> p pair c v", c=2, v=33)
            for (r0, rn, po) in [(0, 33, 0), (33, 32, 96 * M)]:
                for (cj, co) in [(0, 0), (1, 95)]:
                    dst = bass.AP(outd, po + co,
                                  [[M, rn], [M * M, NPAIR], [1, 33]])
                    nc.sync.dma_start(dst, ogp[r0:r0 + rn, :, cj])

    nc.compile()
    return nc


def _host_tables(la1, la2, shifted):
    """Per-core (offs [1,NPK*4] int32, mask-gather source indices [NPAIR])."""
    la1 = np.concatenate([la1.astype(np.int64), [80]])     # phi pair appended
    la2 = np.concatenate([la2.astype(np.int64), [80]])
    sh = np.concatenate([shifted.astype(np.int64), [5]])
    offs, mgi = [], []
    for (st, cnt) in QUARTS:
        t = np.zeros((NPAIR, 2), np.int64)
        mi = np.zeros(NPAIR, np.int64)
        idx = np.arange(st, st + cnt)
        t[:cnt, 0] = la1[idx] * 512
        t[:cnt, 1] = la2[idx] * 512
        mi[:cnt] = sh[idx]
        o = np.zeros((NPK, 4), np.int32)
        o[:, 0] = t[0::2, 0]
        o[:, 1] = t[0::2, 1]
        o[:, 2] = t[1::2, 0]
        o[:, 3] = t[1::2, 1]
        offs.append(o.reshape(1, -1))
        mgi.append(mi)
    return offs, mgi


def _host_consts():
    k = np.arange(M)
    W = np.exp(-2j * np.pi * np.outer(k, k) / M)
    Wr = W.real.astype(np.float32)
    Wi = W.imag.astype(np.float32)
    V = np.conj(W) / M
    Ar = V.real.astype(np.float32)
    Ai = V.imag.astype(np.float32)
    Pr = Ar / 16384.0
    Pi = Ai / 16384.0
    blocks = [Wr, Wi, -Wi, Wr,            # 0-3: fwd stage1 + hatx stage2
              Ar, Ai, -Ai, Ar,            # 4-7: inverse stages
              Wr, Wi, Wi, -Wr,            # 8-11: fwd stage2 rhs1 (512)
              -Wi, Wr, Wr, Wi]            # 12-15: fwd stage2 rhs2 (512)
    wmat = np.concatenate(blocks, axis=1).astype(ml_dtypes.bfloat16)
    sa = np.r_[0:33, 96:128]
    sb = np.r_[0:33, 95:128]
    wpair = np.concatenate(
        [Ar[:, sa], Ai[:, sa], -Ai[:, sa], Ar[:, sa],
         Pr[:, sb], Pi[:, sb], -Pi[:, sb], Pr[:, sb],
         -Ar[:, sa], -Ai[:, sa]], axis=1
    ).astype(ml_dtypes.bfloat16)
    cns = np.zeros((M, 260), np.float32)
    cns[:, 0] = 1.0
    cns[:, 2:130] = np.eye(M, dtype=np.float32)
    cns[:, 130:258] = 1.0
    return wmat, wpair, cns


def _prepare(inputs):
    x = np.asarray(inputs["x"], np.float32)
    hatpsi = np.asarray(inputs["hatpsi"], np.float32)
    hatphi = np.asarray(inputs["hatphi"], np.float32)
    masks = np.asarray(inputs["masks_shift"], np.float32)
    la1 = np.asarray(inputs["la1"])
    la2 = np.asarray(inputs["la2"])
    shifted = np.asarray(inputs["shifted"])

    wmat, wpair, cns = _host_consts()
    psi = np.concatenate(
        [hatpsi.transpose(2, 0, 1, 3).reshape(M, 20 * M), hatphi], axis=1
    ).astype(ml_dtypes.bfloat16)
    offs, mgi = _host_tables(la1, la2, shifted)
    masks_bf = masks.astype(ml_dtypes.bfloat16)
    sa = np.r_[0:33, 96:128]
    sb = np.r_[0:33, 95:128]

    in_maps = []
    for c in range(8):
        b, q = c // 4, c % 4
        mg = masks_bf[mgi[q]]                       # [NPAIR,128,128]
        mg = mg[:, sa][:, :, sb]                    # [NPAIR,65,66]
        mg = np.ascontiguousarray(mg.transpose(1, 0, 2)).reshape(65, NPAIR * 66)
        in_maps.append({
            "xin": np.ascontiguousarray(x[b, 0]),
            "psi": psi,
            "wmat": wmat,
            "wpair": wpair,
            "mgath": mg,
            "cns": cns,
            "offs": offs[q],
        })
    return in_maps


def _assemble(results):
    out = np.empty((2, P_TOT, M * M), np.float32)
    for c in range(8):
        b, q = c // 4, c % 4
        s, cnt = QUARTS[q]
        r = results[c]["out"].reshape(NPAIR, M * M)
        out[b, s:s + cnt] = r[:cnt].astype(np.float32)
    return out.reshape(2, -1)


def kernel(**inputs):
    if "nc" not in _cache:
        _cache["nc"] = _build_nc()
    nc = _cache["nc"]
    in_maps = _prepare(inputs)
    res = run_bass_kernel_spmd(nc, in_maps, core_ids=list(range(8)))
    return _assemble(res.results)


def kernel_traced(tmpdir=None, **inputs):
    """Like kernel() but with neuron-profile tracing; returns (out, results)."""
    if "nc" not in _cache:
        _cache["nc"] = _build_nc()
    nc = _cache["nc"]
    in_maps = _prepare(inputs)
    res = run_bass_kernel_spmd(
        nc, in_maps, core_ids=list(range(8)), trace=True, tmpdir=tmpdir
    )
    return _assemble(res.results), res


# revision 10
# speedup vs baseline: 1.1515x; 1.1482x over previous
"""ALPHA scattering-covariance kernel for 8 Trainium2 NeuronCores.

Math (per batch element b, nc=1, M=N=128, J=5 L=4 A=4, P=960 pairs + 1 phi):
  hatx = fft2(x)
  z_jl = ifft2(hatx * hatpsi[j,l])           (20 complex fields)
  field k=4*jl+a: u = relu(phase_a(z_jl)),   phase in {Re, -Im, -Re, +Im}
  phi field (k=80): u = Re(ifft2(hatx*hatphi))   (no relu)
  n_k = (u - mean)/std ;  hat_n_k = fft2(n_k)
  pair p: corr = Re(ifft2(hat_n[la1] * conj(hat_n[la2]))) / (M*N) * masks[shifted]
  out[b] = concat(pairs 0..959, phi autocorr)

2D FFTs run as transpose-matmuls on the TensorEngine: tmm(P, Q) := P^T @ Q with
P stationary; fft2(n) = W n W (W symmetric), two chained tmms absorb the
inter-stage transposes. Pairs are processed two-per-slot packed into one
complex IFFT (both spectra Hermitian, so ifft2(H_p + i H_q) = corr_p + i corr_q).

Normalization is folded into the spectral domain: fft2((u-mu)/std) =
fft2(u)/std with the DC coefficient forced to 0 (mean-sub only affects DC).
So the raw field FFTs run unscaled, the PSUM->SBUF copy applies 1/std, and a
single strided memset zeroes the DC entries of all 81 spectra.

The per-pair combine Zr = Hr_p - Hi_q, Zi = Hi_p + Hr_q is absorbed into the
first IFFT stage's PSUM accumulation (4 matmuls against +/- copies of the
stage-A weights) instead of DVE ops -- DVE is the pair-loop bottleneck.

Sharding: core c = batch (c//4) x pair-quarter (c%4). Spectra of all 81 fields
live in SBUF as (re|im|im|-re) 512-blocks; per-pair operands are fetched with
register-offset dynamic APs (offsets la*512 from an int32 input table), so one
static graph serves all 8 cores.
"""

import numpy as np
import ml_dtypes

import concourse.bass as bass
import concourse.bacc as bacc
import concourse.tile as tile
import concourse.mybir as mybir
from concourse.bass_utils import run_bass_kernel_spmd

BF16 = mybir.dt.bfloat16
F32 = mybir.dt.float32
I32 = mybir.dt.int32
AF = mybir.ActivationFunctionType
OP = mybir.AluOpType
DVE = mybir.EngineType.DVE

M = 128
NJL = 21          # 20 (j,l) filters + phi as slot 20
NFLD = 81         # 80 alpha fields + normalized phi field as field 80
NPK = 121         # packed pair slots per core (2 pairs each)
NPAIR = 242       # padded pairs per core
QUARTS = [(0, 241), (241, 240), (481, 240), (721, 240)]  # (start, count) of 961
P_TOT = 961

_cache = {}


def _build_nc():
    nc = bacc.Bacc("TRN2", target_bir_lowering=False, debug=False, num_devices=8)

    xin = nc.dram_tensor("xin", [M, M], F32, kind="ExternalInput")
    psid = nc.dram_tensor("psi", [M, NJL * M], BF16, kind="ExternalInput")
    wmatd = nc.dram_tensor("wmat", [M, 16 * M], BF16, kind="ExternalInput")
    wpaird = nc.dram_tensor("wpair", [M, 654], BF16, kind="ExternalInput")
    mgd = nc.dram_tensor("mgath", [65, NPK * 132], BF16, kind="ExternalInput")
    cnsd = nc.dram_tensor("cns", [M, 260], F32, kind="ExternalInput")
    offsd = nc.dram_tensor("offs", [1, NPK * 4], I32, kind="ExternalInput")
    outd = nc.dram_tensor("out", [NPAIR, M, M], BF16, kind="ExternalOutput")

    with tile.TileContext(nc) as tc:
        with (
            tc.tile_pool(name="const", bufs=1) as cp,
            tc.tile_pool(name="work", bufs=3) as wp,
            tc.tile_pool(name="pairw", bufs=4) as pw,
            tc.tile_pool(name="ps256", bufs=2, space="PSUM") as pp,
            tc.tile_pool(name="ps512", bufs=2, space="PSUM") as pv,
            tc.tile_pool(name="psB", bufs=4, space="PSUM") as pq,
        ):
            # ---- constants into SBUF ----
            wsb = cp.tile([M, 16 * M], BF16)
            nc.sync.dma_start(wsb, wmatd.ap())
            wpsb = cp.tile([M, 654], BF16)
            nc.sync.dma_start(wpsb, wpaird.ap())
            psisb = cp.tile([M, NJL * M], BF16)
            nc.sync.dma_start(psisb, psid.ap())
            MG = cp.tile([65, NPK * 132], BF16)
            nc.sync.dma_start(MG, mgd.ap())
            cns = cp.tile([M, 260], F32)
            nc.sync.dma_start(cns, cnsd.ap())
            offsb = cp.tile([1, NPK * 4], I32)
            nc.sync.dma_start(offsb, offsd.ap())
            xf = cp.tile([M, M], F32)
            nc.sync.dma_start(xf, xin.ap())

            # zero-fill the never-written regions of out: mid rows 33..95 and
            # dead cols 33..94 of the live rows. Few big DMAs, 4 queue-chunks.
            zsb = cp.tile([M, M], BF16)
            nc.vector.memzero(zsb)
            for j0 in range(0, NPAIR, 61):
                n = min(61, NPAIR - j0)
                # rows 33..95 full width
                dst = bass.AP(outd, j0 * M * M + 33 * M,
                              [[M, 63], [M * M, n], [1, M]])
                src = zsb[0:63, 0:M].rearrange("p (one x) -> p one x", one=1)
                nc.sync.dma_start(dst, src.to_broadcast((63, n, M)))
                # rows 0..32, cols 33..94
                dst = bass.AP(outd, j0 * M * M + 33,
                              [[M, 33], [M * M, n], [1, 62]])
                src = zsb[0:33, 0:62].rearrange("p (one x) -> p one x", one=1)
                nc.sync.dma_start(dst, src.to_broadcast((33, n, 62)))
                # rows 96..127, cols 33..94
                dst = bass.AP(outd, j0 * M * M + 96 * M + 33,
                              [[M, 32], [M * M, n], [1, 62]])
                src = zsb[0:32, 0:62].rearrange("p (one x) -> p one x", one=1)
                nc.sync.dma_start(dst, src.to_broadcast((32, n, 62)))

            # big bf16 staging buffer for all live-window outputs
            OGB = cp.tile([65, NPK * 132], BF16)

            def WB(i, n=1):
                return wsb[:, i * M:(i + n) * M]

            ones_col = cns[:, 0:1]          # [128,1] f32 of 1.0
            ones_row = cns[0:1, 130:258]    # [1,128] f32 of 1.0
            ident = cns[:, 2:130]           # [128,128] f32 identity

            # persistent SBUF state
            SPBIG = cp.tile([M, NFLD * 512], BF16)  # (re|im|im|-re) per field
            FLDS = cp.tile([M, NFLD * M], BF16)     # relu'd fields
            HX = cp.tile([M, 256], BF16)            # hatx (re|im)
            SUMS = cp.tile([M, NFLD], F32)
            SQS = cp.tile([M, NFLD], F32)
            AB = cp.tile([M, NFLD], F32)            # per-field 1/std bcast

            # ---- hatx = fft2(x) ----
            xb = wp.tile([M, M], BF16, tag="xb")
            nc.scalar.activation(xb, xf, AF.Copy)
            psF = pp.tile([M, 256], F32, tag="ps256")
            nc.tensor.matmul(psF, xb, WB(0, 2), start=True, stop=True)
            fsb = wp.tile([M, 256], BF16, tag="fsb")
            nc.scalar.activation(fsb, psF, AF.Copy)
            psH = pp.tile([M, 256], F32, tag="ps256")
            nc.tensor.matmul(psH, fsb[:, 0:M], WB(0, 2), start=True, stop=False)
            nc.tensor.matmul(psH, fsb[:, M:256], WB(2, 2), start=False, stop=True)
            nc.scalar.activation(HX, psH, AF.Copy)

            # ---- z_jl = ifft2(hatx * psi_jl); fields + row-sums ----
            HX3 = HX[:, 0:256].rearrange("p (two c) -> p two c", two=2)
            for jl in range(NJL):
                pj = psisb[:, jl * M:(jl + 1) * M]
                pj3 = pj.rearrange("p (one c) -> p one c", one=1)
                ab2 = wp.tile([M, 256], BF16, tag="ab2")
                nc.vector.tensor_tensor(
                    ab2.rearrange("p (two c) -> p two c", two=2),
                    HX3, pj3.to_broadcast((M, 2, M)), OP.mult)
                psT = pp.tile([M, 256], F32, tag="ps256")
                nc.tensor.matmul(psT, ab2[:, 0:M], WB(4, 2), start=True, stop=False)
                nc.tensor.matmul(psT, ab2[:, M:256], WB(6, 2), start=False, stop=True)
                tsb = wp.tile([M, 256], BF16, tag="tsb")
                if jl % 2 == 0:
                    nc.scalar.activation(tsb, psT, AF.Copy)
                else:
                    nc.vector.tensor_scalar(tsb, psT, 1.0, None, OP.mult)
                psZ = pp.tile([M, 256], F32, tag="ps256")
                nc.tensor.matmul(psZ, tsb[:, 0:M], WB(4, 2), start=True, stop=False)
                nc.tensor.matmul(psZ, tsb[:, M:256], WB(6, 2), start=False, stop=True)
                if jl < 20:
                    # fields 4jl+a: relu(+zr), relu(-zi), relu(-zr), relu(+zi)
                    # a=0,1 on ACT (relu + fused row-sum), a=2,3 on DVE
                    for a, (half, sc) in enumerate(
                        [(0, 1.0), (1, -1.0), (0, -1.0), (1, 1.0)]
                    ):
                        k = 4 * jl + a
                        fld = FLDS[:, k * M:(k + 1) * M]
                        src = psZ[:, half * M:(half + 1) * M]
                        if a < 2:
                            nc.scalar.activation(
                                fld, src, AF.Relu,
                                scale=sc, accum_out=SUMS[:, k:k + 1],
                            )
                        else:
                            nc.vector.tensor_scalar(
                                fld, src, sc, 0.0, OP.mult, OP.max)
                            nc.vector.tensor_reduce(
                                SUMS[:, k:k + 1], fld, mybir.AxisListType.X, OP.add)
                    # squared sums: a=0,1 on ACT Square, a=2,3 on DVE
                    for a in range(4):
                        k = 4 * jl + a
                        fld = FLDS[:, k * M:(k + 1) * M]
                        dump = wp.tile([M, M], BF16, tag="dump")
                        if a < 2:
                            nc.scalar.activation(
                                dump, fld, AF.Square, accum_out=SQS[:, k:k + 1])
                        else:
                            nc.vector.tensor_tensor(dump, fld, fld, OP.mult)
                            nc.vector.tensor_reduce(
                                SQS[:, k:k + 1], dump, mybir.AxisListType.X, OP.add)
                else:
                    k = 80
                    fld = FLDS[:, k * M:(k + 1) * M]
                    nc.scalar.activation(
                        fld, psZ[:, 0:M], AF.Copy,
                        accum_out=SUMS[:, k:k + 1],
                    )
                    dump = wp.tile([M, M], BF16, tag="dump")
                    nc.scalar.activation(
                        dump, fld, AF.Square, accum_out=SQS[:, k:k + 1])

            # ---- stats: total mean/var -> alpha = 1/std, bcast to [128, 81] ----
            psS = pp.tile([M, 256], F32, tag="ps256")
            nc.tensor.matmul(psS[0:NFLD, 0:1], SUMS, ones_col, start=True, stop=True)
            nc.tensor.matmul(psS[0:NFLD, 1:2], SQS, ones_col, start=True, stop=True)
            st = wp.tile([NFLD, 2], F32, tag="st1")
            nc.scalar.activation(st, psS[0:NFLD, 0:2], AF.Copy, scale=1.0 / 16384.0)
            mu = st[:, 0:1]
            e2 = st[:, 1:2]
            mu2 = wp.tile([NFLD, 1], F32, tag="mu2")
            nc.vector.tensor_tensor(mu2, mu, mu, OP.mult)
            var = wp.tile([NFLD, 1], F32, tag="var")
            nc.vector.tensor_tensor(var, e2, mu2, OP.subtract)
            alph = wp.tile([NFLD, 1], F32, tag="alph")
            sdev = wp.tile([NFLD, 1], F32, tag="sdev")
            nc.scalar.activation(sdev, var, AF.Sqrt)
            nc.vector.reciprocal(alph, sdev)
            psS2 = pp.tile([M, 256], F32, tag="ps256")
            nc.tensor.transpose(psS2[0:1, 0:NFLD], alph, ident[0:NFLD, 0:NFLD])
            arow = wp.tile([1, NFLD], F32, tag="arow")
            nc.scalar.activation(arow, psS2[0:1, 0:NFLD], AF.Copy)
            psAB = pp.tile([M, 256], F32, tag="ps256")
            nc.tensor.matmul(psAB[:, 0:NFLD], ones_row, arow, start=True, stop=True)
            nc.scalar.activation(AB, psAB[:, 0:NFLD], AF.Copy)

            # ---- forward fft per field (unnormalized) -> SPBIG * (1/std) ----
            for k in range(NFLD):
                fld = FLDS[:, k * M:(k + 1) * M]
                psF2 = pp.tile([M, 256], F32, tag="ps256")
                nc.tensor.matmul(psF2, fld, WB(0, 2), start=True, stop=True)
                f2 = wp.tile([M, 256], BF16, tag="fsb")
                if k % 2 == 0:
                    nc.scalar.activation(f2, psF2, AF.Copy)
                else:
                    nc.vector.tensor_scalar(f2, psF2, 1.0, None, OP.mult)
                psH5 = pv.tile([M, 512], F32, tag="ps512")
                nc.tensor.matmul(psH5, f2[:, 0:M], WB(8, 4), start=True, stop=False)
                nc.tensor.matmul(psH5, f2[:, M:256], WB(12, 4), start=False, stop=True)
                dst = SPBIG[:, k * 512:(k + 1) * 512]
                if k % 2 == 0:
                    nc.vector.tensor_scalar(dst, psH5, AB[:, k:k + 1], None, OP.mult)
                else:
                    nc.scalar.activation(dst, psH5, AF.Copy, scale=AB[:, k:k + 1])
            # mean-sub == zero the DC entry of every spectrum (all 4 sub-blocks)
            spz = SPBIG[:].rearrange("p (k c) -> p k c", c=512)
            for off in range(0, 512, 128):
                nc.vector.memset(spz[0:1, :, off:off + 1], 0.0)

            # ---- packed pair loop: slot s covers pairs (2s, 2s+1) ----
            vals = None
            psB = None
            for s in range(NPK):
                if s % 8 == 0:
                    nv = min(8, NPK - s) * 4
                    _, vals = nc.values_load_multi_w_load_instructions(
                        offsb[0:1, 4 * s:4 * s + nv],
                        engines=[DVE],
                        min_val=0, max_val=80 * 512,
                        skip_runtime_bounds_check=True,
                    )
                o1p, o2p, o1q, o2q = vals[4 * (s % 8):4 * (s % 8) + 4]

                # products: (t1|t2|t3|-t4) per pair; lhs is the (re|im|im|-re)
                # block of la1, rhs the (re|im) half of la2 repeated twice
                prod = pw.tile([M, 1024], BF16, tag="prod")
                rhs_p = SPBIG[:, bass.ds(o2p, 256)].rearrange(
                    "p (one c) -> p one c", one=1).to_broadcast((M, 2, 256))
                rhs_q = SPBIG[:, bass.ds(o2q, 256)].rearrange(
                    "p (one c) -> p one c", one=1).to_broadcast((M, 2, 256))
                lhs_p = SPBIG[:, bass.ds(o1p, 512)].rearrange(
                    "p (two c) -> p two c", two=2)
                lhs_q = SPBIG[:, bass.ds(o1q, 512)].rearrange(
                    "p (two c) -> p two c", two=2)
                nc.vector.tensor_tensor(
                    prod[:, 0:512].rearrange("p (two c) -> p two c", two=2),
                    lhs_p, rhs_p, OP.mult)
                nc.vector.tensor_tensor(
                    prod[:, 512:1024].rearrange("p (two c) -> p two c", two=2),
                    lhs_q, rhs_q, OP.mult)

                # X = (Hr_p|Hi_p|Hr_q|Hi_q): pairwise sums of adjacent blocks
                X = pw.tile([M, 512], BF16, tag="X")
                pr3 = prod[:].rearrange("p (n two c) -> p n two c", two=2, c=M)
                xeng = nc.gpsimd if s % 4 != 1 else nc.vector
                xeng.tensor_tensor(
                    X[:].rearrange("p (n c) -> p n c", c=M),
                    pr3[:, :, 0, :], pr3[:, :, 1, :], OP.add)

                # packed ifft stage A with the Zr/Zi +/- combine folded into
                # PSUM accumulation: Zr = X0 - X3 (wA0), Zi = X1 + X2 (wA1)
                psA = pv.tile([M, 130], F32, tag="ps512",
                              padded_shape=[M, 512])
                nc.tensor.matmul(psA, X[:, 0:M], wpsb[:, 0:130], start=True, stop=False)
                nc.tensor.matmul(psA, X[:, M:256], wpsb[:, 130:260], start=False, stop=False)
                nc.tensor.matmul(psA, X[:, 256:384], wpsb[:, 130:260], start=False, stop=False)
                nc.tensor.matmul(psA, X[:, 384:512], wpsb[:, 524:654], start=False, stop=True)
                t1sb = pw.tile([M, 130], BF16, tag="t1sb")
                nc.scalar.activation(t1sb, psA, AF.Copy)

                # stage B batched 3 slots per PSUM bank; one masked copy-out
                g, r = divmod(s, 3)
                if r == 0:
                    psB = pq.tile([65, 396], F32, tag="psB")
                pBs = psB[:, r * 132:(r + 1) * 132]
                nc.tensor.matmul(pBs, t1sb[:, 0:65], wpsb[:, 260:392], start=True, stop=False)
                nc.tensor.matmul(pBs, t1sb[:, 65:130], wpsb[:, 392:524], start=False, stop=True)
                if r == 2 or s == NPK - 1:
                    s0 = 3 * g
                    w = (r + 1) * 132
                    nc.vector.tensor_tensor(
                        OGB[:, 132 * s0:132 * s0 + w], psB[:, 0:w],
                        MG[:, 132 * s0:132 * s0 + w], OP.mult)

            # 4 big output DMAs: (rows 0..32 | 96..127) x (cols 0..32 | 95..127)
            ogp = OGB[:].rearrange("p (pair c v) -> p pair c v", c=2, v=33)
            for (r0, rn, po) in [(0, 33, 0), (33, 32, 96 * M)]:
                for (cj, co) in [(0, 0), (1, 95)]:
                    dst = bass.AP(outd, po + co,
                                  [[M, rn], [M * M, NPAIR], [1, 33]])
                    nc.sync.dma_start(dst, ogp[r0:r0 + rn, :, cj])

    nc.compile()
    return nc


def _host_tables(la1, la2, shifted):
    """Per-core (offs [1,NPK*4] int32, mask-gather source indices [NPAIR])."""
    la1 = np.concatenate([la1.astype(np.int64), [80]])     # phi pair appended
    la2 = np.concatenate([la2.astype(np.int64), [80]])
    sh = np.concatenate([shifted.astype(np.int64), [5]])
    offs, mgi = [], []
    for (st, cnt) in QUARTS:
        t = np.zeros((NPAIR, 2), np.int64)
        mi = np.zeros(NPAIR, np.int64)
        idx = np.arange(st, st + cnt)
        t[:cnt, 0] = la1[idx] * 512
        t[:cnt, 1] = la2[idx] * 512
        mi[:cnt] = sh[idx]
        o = np.zeros((NPK, 4), np.int32)
        o[:, 0] = t[0::2, 0]
        o[:, 1] = t[0::2, 1]
        o[:, 2] = t[1::2, 0]
        o[:, 3] = t[1::2, 1]
        offs.append(o.reshape(1, -1))
        mgi.append(mi)
    return offs, mgi


def _host_consts():
    k = np.arange(M)
    W = np.exp(-2j * np.pi * np.outer(k, k) / M)
    Wr = W.real.astype(np.float32)
    Wi = W.imag.astype(np.float32)
    V = np.conj(W) / M
    Ar = V.real.astype(np.float32)
    Ai = V.imag.astype(np.float32)
    Pr = Ar / 16384.0
    Pi = Ai / 16384.0
    blocks = [Wr, Wi, -Wi, Wr,            # 0-3: fwd stage1 + hatx stage2
              Ar, Ai, -Ai, Ar,            # 4-7: inverse stages
              Wr, Wi, Wi, -Wr,            # 8-11: fwd stage2 rhs1 (512)
              -Wi, Wr, Wr, Wi]            # 12-15: fwd stage2 rhs2 (512)
    wmat = np.concatenate(blocks, axis=1).astype(ml_dtypes.bfloat16)
    sa = np.r_[0:33, 96:128]
    sb = np.r_[0:33, 95:128]
    wpair = np.concatenate(
        [Ar[:, sa], Ai[:, sa], -Ai[:, sa], Ar[:, sa],
         Pr[:, sb], Pi[:, sb], -Pi[:, sb], Pr[:, sb],
         -Ar[:, sa], -Ai[:, sa]], axis=1
    ).astype(ml_dtypes.bfloat16)
    cns = np.zeros((M, 260), np.float32)
    cns[:, 0] = 1.0
    cns[:, 2:130] = np.eye(M, dtype=np.float32)
    cns[:, 130:258] = 1.0
    return wmat, wpair, cns


def _prepare(inputs):
    x = np.asarray(inputs["x"], np.float32)
    hatpsi = np.asarray(inputs["hatpsi"], np.float32)
    hatphi = np.asarray(inputs["hatphi"], np.float32)
    masks = np.asarray(inputs["masks_shift"], np.float32)
    la1 = np.asarray(inputs["la1"])
    la2 = np.asarray(inputs["la2"])
    shifted = np.asarray(inputs["shifted"])

    wmat, wpair, cns = _host_consts()
    psi = np.concatenate(
        [hatpsi.transpose(2, 0, 1, 3).reshape(M, 20 * M), hatphi], axis=1
    ).astype(ml_dtypes.bfloat16)
    offs, mgi = _host_tables(la1, la2, shifted)
    masks_bf = masks.astype(ml_dtypes.bfloat16)
    sa = np.r_[0:33, 96:128]
    sb = np.r_[0:33, 95:128]

    in_maps = []
    for c in range(8):
        b, q = c // 4, c % 4
        mg = masks_bf[mgi[q]]                       # [NPAIR,128,128]
        mg = mg[:, sa][:, :, sb]                    # [NPAIR,65,66]
        mg = np.ascontiguousarray(mg.transpose(1, 0, 2)).reshape(65, NPAIR * 66)
        in_maps.append({
            "xin": np.ascontiguousarray(x[b, 0]),
            "psi": psi,
            "wmat": wmat,
            "wpair": wpair,
            "mgath": mg,
            "cns": cns,
            "offs": offs[q],
        })
    return in_maps


def _assemble(results):
    out = np.empty((2, P_TOT, M * M), np.float32)
    for c in range(8):
        b, q = c // 4, c % 4
        s, cnt = QUARTS[q]
        r = results[c]["out"].reshape(NPAIR, M * M)
        out[b, s:s + cnt] = r[:cnt].astype(np.float32)
    return out.reshape(2, -1)


def kernel(**inputs):
    if "nc" not in _cache:
        _cache["nc"] = _build_nc()
    nc = _cache["nc"]
    in_maps = _prepare(inputs)
    res = run_bass_kernel_spmd(nc, in_maps, core_ids=list(range(8)))
    return _assemble(res.results)


def kernel_traced(tmpdir=None, **inputs):
    """Like kernel() but with neuron-profile tracing; returns (out, results)."""
    if "nc" not in _cache:
        _cache["nc"] = _build_nc()
    nc = _cache["nc"]
    in_maps = _prepare(inputs)
    res = run_bass_kernel_spmd(
        nc, in_maps, core_ids=list(range(8)), trace=True, tmpdir=tmpdir
    )
    return _assemble(res.results), res


# revision 13
# speedup vs baseline: 1.1944x; 1.0373x over previous
"""ALPHA scattering-covariance kernel for 8 Trainium2 NeuronCores.

Math (per batch element b, nc=1, M=N=128, J=5 L=4 A=4, P=960 pairs + 1 phi):
  hatx = fft2(x)
  z_jl = ifft2(hatx * hatpsi[j,l])           (20 complex fields)
  field k=4*jl+a: u = relu(phase_a(z_jl)),   phase in {Re, -Im, -Re, +Im}
  phi field (k=80): u = Re(ifft2(hatx*hatphi))   (no relu)
  n_k = (u - mean)/std ;  hat_n_k = fft2(n_k)
  pair p: corr = Re(ifft2(hat_n[la1] * conj(hat_n[la2]))) / (M*N) * masks[shifted]
  out[b] = concat(pairs 0..959, phi autocorr)

2D FFTs run as transpose-matmuls on the TensorEngine: tmm(P, Q) := P^T @ Q with
P stationary; fft2(n) = W n W (W symmetric), two chained tmms absorb the
inter-stage transposes. Pairs are processed two-per-slot packed into one
complex IFFT (both spectra Hermitian, so ifft2(H_p + i H_q) = corr_p + i corr_q).

Normalization is folded into the spectral domain: fft2((u-mu)/std) =
fft2(u)/std with the DC coefficient forced to 0 (mean-sub only affects DC).
So the raw field FFTs run unscaled, the PSUM->SBUF copy applies 1/std, and a
single strided memset zeroes the DC entries of all 81 spectra.

The per-pair combine Zr = Hr_p - Hi_q, Zi = Hi_p + Hr_q is absorbed into the
first IFFT stage's PSUM accumulation (4 matmuls against +/- copies of the
stage-A weights) instead of DVE ops -- DVE is the pair-loop bottleneck.

Sharding: core c = batch (c//4) x pair-quarter (c%4). Spectra of all 81 fields
live in SBUF as (re|im|im|-re) 512-blocks; per-pair operands are fetched with
register-offset dynamic APs (offsets la*512 from an int32 input table), so one
static graph serves all 8 cores.
"""

import numpy as np
import ml_dtypes

import concourse.bass as bass
import concourse.bacc as bacc
import concourse.tile as tile
import concourse.mybir as mybir
from concourse.bass_utils import run_bass_kernel_spmd

BF16 = mybir.dt.bfloat16
F32 = mybir.dt.float32
I32 = mybir.dt.int32
AF = mybir.ActivationFunctionType
OP = mybir.AluOpType
DVE = mybir.EngineType.DVE

M = 128
NJL = 21          # 20 (j,l) filters + phi as slot 20
NFLD = 81         # 80 alpha fields + normalized phi field as field 80
NPK = 121         # packed pair slots per core (2 pairs each)
NPAIR = 242       # padded pairs per core
QUARTS = [(0, 241), (241, 240), (481, 240), (721, 240)]  # (start, count) of 961
P_TOT = 961

_cache = {}


def _build_nc():
    nc = bacc.Bacc("TRN2", target_bir_lowering=False, debug=False, num_devices=8)

    xin = nc.dram_tensor("xin", [M, M], F32, kind="ExternalInput")
    psid = nc.dram_tensor("psi", [M, NJL * M], BF16, kind="ExternalInput")
    wmatd = nc.dram_tensor("wmat", [M, 16 * M], BF16, kind="ExternalInput")
    wpaird = nc.dram_tensor("wpair", [M, 654], BF16, kind="ExternalInput")
    mgd = nc.dram_tensor("mgath", [65, NPK * 132], BF16, kind="ExternalInput")
    cnsd = nc.dram_tensor("cns", [M, 260], F32, kind="ExternalInput")
    offsd = nc.dram_tensor("offs", [1, NPK * 3], I32, kind="ExternalInput")
    outd = nc.dram_tensor("out", [NPAIR, M, M], BF16, kind="ExternalOutput")

    with tile.TileContext(nc) as tc:
        with (
            tc.tile_pool(name="const", bufs=1) as cp,
            tc.tile_pool(name="work", bufs=3) as wp,
            tc.tile_pool(name="pairw", bufs=4) as pw,
            tc.tile_pool(name="ps256", bufs=2, space="PSUM") as pp,
            tc.tile_pool(name="ps512", bufs=2, space="PSUM") as pv,
            tc.tile_pool(name="psB", bufs=4, space="PSUM") as pq,
        ):
            # ---- constants into SBUF ----
            wsb = cp.tile([M, 16 * M], BF16)
            nc.sync.dma_start(wsb, wmatd.ap())
            wpsb = cp.tile([M, 654], BF16)
            nc.sync.dma_start(wpsb, wpaird.ap())
            psisb = cp.tile([M, NJL * M], BF16)
            nc.sync.dma_start(psisb, psid.ap())
            MG = cp.tile([65, NPK * 132], BF16)
            nc.sync.dma_start(MG, mgd.ap())
            cns = cp.tile([M, 260], F32)
            nc.sync.dma_start(cns, cnsd.ap())
            offsb = cp.tile([1, NPK * 3], I32)
            nc.sync.dma_start(offsb, offsd.ap())
            xf = cp.tile([M, M], F32)
            nc.sync.dma_start(xf, xin.ap())

            # zero-fill the never-written regions of out: mid rows 33..95 and
            # dead cols 33..94 of the live rows. Few big DMAs, 4 queue-chunks.
            zsb = cp.tile([M, M], BF16)
            nc.vector.memzero(zsb)
            for j0 in range(0, NPAIR, 61):
                n = min(61, NPAIR - j0)
                # rows 33..95 full width
                dst = bass.AP(outd, j0 * M * M + 33 * M,
                              [[M, 63], [M * M, n], [1, M]])
                src = zsb[0:63, 0:M].rearrange("p (one x) -> p one x", one=1)
                nc.sync.dma_start(dst, src.to_broadcast((63, n, M)))
                # rows 0..32, cols 33..94
                dst = bass.AP(outd, j0 * M * M + 33,
                              [[M, 33], [M * M, n], [1, 62]])
                src = zsb[0:33, 0:62].rearrange("p (one x) -> p one x", one=1)
                nc.sync.dma_start(dst, src.to_broadcast((33, n, 62)))
                # rows 96..127, cols 33..94
                dst = bass.AP(outd, j0 * M * M + 96 * M + 33,
                              [[M, 32], [M * M, n], [1, 62]])
                src = zsb[0:32, 0:62].rearrange("p (one x) -> p one x", one=1)
                nc.sync.dma_start(dst, src.to_broadcast((32, n, 62)))

            # big bf16 staging buffer for all live-window outputs
            OGB = cp.tile([65, NPK * 132], BF16)

            def WB(i, n=1):
                return wsb[:, i * M:(i + n) * M]

            ones_col = cns[:, 0:1]          # [128,1] f32 of 1.0
            ones_row = cns[0:1, 130:258]    # [1,128] f32 of 1.0
            ident = cns[:, 2:130]           # [128,128] f32 identity

            # persistent SBUF state
            SPBIG = cp.tile([M, NFLD * 512], BF16)  # (re|im|im|-re) per field
            FLDS = cp.tile([M, NFLD * M], BF16)     # relu'd fields
            HX = cp.tile([M, 256], BF16)            # hatx (re|im)
            SUMS = cp.tile([M, NFLD], F32)
            SQS = cp.tile([M, NFLD], F32)
            AB = cp.tile([M, NFLD], F32)            # per-field 1/std bcast

            # ---- hatx = fft2(x) ----
            xb = wp.tile([M, M], BF16, tag="xb")
            nc.scalar.activation(xb, xf, AF.Copy)
            psF = pp.tile([M, 256], F32, tag="ps256")
            nc.tensor.matmul(psF, xb, WB(0, 2), start=True, stop=True)
            fsb = wp.tile([M, 256], BF16, tag="fsb")
            nc.scalar.activation(fsb, psF, AF.Copy)
            psH = pp.tile([M, 256], F32, tag="ps256")
            nc.tensor.matmul(psH, fsb[:, 0:M], WB(0, 2), start=True, stop=False)
            nc.tensor.matmul(psH, fsb[:, M:256], WB(2, 2), start=False, stop=True)
            nc.scalar.activation(HX, psH, AF.Copy)

            # ---- z_jl = ifft2(hatx * psi_jl); fields + row-sums ----
            HX3 = HX[:, 0:256].rearrange("p (two c) -> p two c", two=2)
            for jl in range(NJL):
                pj = psisb[:, jl * M:(jl + 1) * M]
                pj3 = pj.rearrange("p (one c) -> p one c", one=1)
                ab2 = wp.tile([M, 256], BF16, tag="ab2")
                nc.vector.tensor_tensor(
                    ab2.rearrange("p (two c) -> p two c", two=2),
                    HX3, pj3.to_broadcast((M, 2, M)), OP.mult)
                psT = pp.tile([M, 256], F32, tag="ps256")
                nc.tensor.matmul(psT, ab2[:, 0:M], WB(4, 2), start=True, stop=False)
                nc.tensor.matmul(psT, ab2[:, M:256], WB(6, 2), start=False, stop=True)
                tsb = wp.tile([M, 256], BF16, tag="tsb")
                if jl % 2 == 0:
                    nc.scalar.activation(tsb, psT, AF.Copy)
                else:
                    nc.vector.tensor_scalar(tsb, psT, 1.0, None, OP.mult)
                psZ = pp.tile([M, 256], F32, tag="ps256")
                nc.tensor.matmul(psZ, tsb[:, 0:M], WB(4, 2), start=True, stop=False)
                nc.tensor.matmul(psZ, tsb[:, M:256], WB(6, 2), start=False, stop=True)
                if jl < 20:
                    # fields 4jl+a: relu(+zr), relu(-zi), relu(-zr), relu(+zi)
                    # a=0,1 on ACT (relu + fused row-sum), a=2,3 on DVE
                    for a, (half, sc) in enumerate(
                        [(0, 1.0), (1, -1.0), (0, -1.0), (1, 1.0)]
                    ):
                        k = 4 * jl + a
                        fld = FLDS[:, k * M:(k + 1) * M]
                        src = psZ[:, half * M:(half + 1) * M]
                        if a < 2:
                            nc.scalar.activation(
                                fld, src, AF.Relu,
                                scale=sc, accum_out=SUMS[:, k:k + 1],
                            )
                        else:
                            nc.vector.tensor_scalar(
                                fld, src, sc, 0.0, OP.mult, OP.max)
                            nc.vector.tensor_reduce(
                                SUMS[:, k:k + 1], fld, mybir.AxisListType.X, OP.add)
                    # squared sums: a=0,1 on ACT Square, a=2,3 on DVE
                    for a in range(4):
                        k = 4 * jl + a
                        fld = FLDS[:, k * M:(k + 1) * M]
                        dump = wp.tile([M, M], BF16, tag="dump")
                        if a < 2:
                            nc.scalar.activation(
                                dump, fld, AF.Square, accum_out=SQS[:, k:k + 1])
                        else:
                            nc.vector.tensor_tensor(dump, fld, fld, OP.mult)
                            nc.vector.tensor_reduce(
                                SQS[:, k:k + 1], dump, mybir.AxisListType.X, OP.add)
                else:
                    k = 80
                    fld = FLDS[:, k * M:(k + 1) * M]
                    nc.scalar.activation(
                        fld, psZ[:, 0:M], AF.Copy,
                        accum_out=SUMS[:, k:k + 1],
                    )
                    dump = wp.tile([M, M], BF16, tag="dump")
                    nc.scalar.activation(
                        dump, fld, AF.Square, accum_out=SQS[:, k:k + 1])

            # ---- stats: total mean/var -> alpha = 1/std, bcast to [128, 81] ----
            psS = pp.tile([M, 256], F32, tag="ps256")
            nc.tensor.matmul(psS[0:NFLD, 0:1], SUMS, ones_col, start=True, stop=True)
            nc.tensor.matmul(psS[0:NFLD, 1:2], SQS, ones_col, start=True, stop=True)
            st = wp.tile([NFLD, 2], F32, tag="st1")
            nc.scalar.activation(st, psS[0:NFLD, 0:2], AF.Copy, scale=1.0 / 16384.0)
            mu = st[:, 0:1]
            e2 = st[:, 1:2]
            mu2 = wp.tile([NFLD, 1], F32, tag="mu2")
            nc.vector.tensor_tensor(mu2, mu, mu, OP.mult)
            var = wp.tile([NFLD, 1], F32, tag="var")
            nc.vector.tensor_tensor(var, e2, mu2, OP.subtract)
            alph = wp.tile([NFLD, 1], F32, tag="alph")
            sdev = wp.tile([NFLD, 1], F32, tag="sdev")
            nc.scalar.activation(sdev, var, AF.Sqrt)
            nc.vector.reciprocal(alph, sdev)
            psS2 = pp.tile([M, 256], F32, tag="ps256")
            nc.tensor.transpose(psS2[0:1, 0:NFLD], alph, ident[0:NFLD, 0:NFLD])
            arow = wp.tile([1, NFLD], F32, tag="arow")
            nc.scalar.activation(arow, psS2[0:1, 0:NFLD], AF.Copy)
            psAB = pp.tile([M, 256], F32, tag="ps256")
            nc.tensor.matmul(psAB[:, 0:NFLD], ones_row, arow, start=True, stop=True)
            nc.scalar.activation(AB, psAB[:, 0:NFLD], AF.Copy)

            # ---- forward fft per field (unnormalized) -> SPBIG * (1/std) ----
            for k in range(NFLD):
                fld = FLDS[:, k * M:(k + 1) * M]
                psF2 = pp.tile([M, 256], F32, tag="ps256")
                nc.tensor.matmul(psF2, fld, WB(0, 2), start=True, stop=True)
                f2 = wp.tile([M, 256], BF16, tag="fsb")
                if k % 2 == 0:
                    nc.scalar.activation(f2, psF2, AF.Copy)
                else:
                    nc.vector.tensor_scalar(f2, psF2, 1.0, None, OP.mult)
                psH5 = pv.tile([M, 512], F32, tag="ps512")
                nc.tensor.matmul(psH5, f2[:, 0:M], WB(8, 4), start=True, stop=False)
                nc.tensor.matmul(psH5, f2[:, M:256], WB(12, 4), start=False, stop=True)
                dst = SPBIG[:, k * 512:(k + 1) * 512]
                if k % 2 == 0:
                    nc.vector.tensor_scalar(dst, psH5, AB[:, k:k + 1], None, OP.mult)
                else:
                    nc.scalar.activation(dst, psH5, AF.Copy, scale=AB[:, k:k + 1])
            # mean-sub == zero the DC entry of every spectrum (all 4 sub-blocks)
            spz = SPBIG[:].rearrange("p (k c) -> p k c", c=512)
            for off in range(0, 512, 128):
                nc.vector.memset(spz[0:1, :, off:off + 1], 0.0)

            # ---- packed pair loop: slot s covers pairs (2s, 2s+1),
            # host-paired so both share la1 (one lhs offset per slot) ----
            vals = None
            psB = None
            for s in range(NPK):
                if s % 8 == 0:
                    nv = min(8, NPK - s) * 3
                    _, vals = nc.values_load_multi_w_load_instructions(
                        offsb[0:1, 3 * s:3 * s + nv],
                        engines=[DVE],
                        min_val=0, max_val=80 * 512,
                        skip_runtime_bounds_check=True,
                    )
                o1, o2p, o2q = vals[3 * (s % 8):3 * (s % 8) + 3]

                # products: (t1|t2|t3|-t4) per pair; lhs is the (re|im|im|-re)
                # block of la1 (shared), rhs the (re|im) half of la2 twice
                prod = pw.tile([M, 1024], BF16, tag="prod")
                rhs_p = SPBIG[:, bass.ds(o2p, 256)].rearrange(
                    "p (one c) -> p one c", one=1).to_broadcast((M, 2, 256))
                rhs_q = SPBIG[:, bass.ds(o2q, 256)].rearrange(
                    "p (one c) -> p one c", one=1).to_broadcast((M, 2, 256))
                lhs = SPBIG[:, bass.ds(o1, 512)].rearrange(
                    "p (two c) -> p two c", two=2)
                nc.vector.tensor_tensor(
                    prod[:, 0:512].rearrange("p (two c) -> p two c", two=2),
                    lhs, rhs_p, OP.mult)
                nc.vector.tensor_tensor(
                    prod[:, 512:1024].rearrange("p (two c) -> p two c", two=2),
                    lhs, rhs_q, OP.mult)

                # X = (Hr_p|Hi_p) for pair p only; pair q's four product
                # blocks go straight into the psA accumulation below
                X = pw.tile([M, 256], BF16, tag="X")
                pr3 = prod[:, 0:512].rearrange("p (n two c) -> p n two c", two=2, c=M)
                xeng = nc.gpsimd if s % 4 != 1 else nc.vector
                xeng.tensor_tensor(
                    X[:].rearrange("p (n c) -> p n c", c=M),
                    pr3[:, :, 0, :], pr3[:, :, 1, :], OP.add)

                # packed ifft stage A; all +/- combines folded into PSUM:
                # psA = Hr_p w0 + Hi_p w1 + (t1q+t2q) w1 + (t3q+t4mq)(-w0)
                psA = pv.tile([M, 130], F32, tag="ps512",
                              padded_shape=[M, 512])
                nc.tensor.matmul(psA, X[:, 0:M], wpsb[:, 0:130], start=True, stop=False)
                nc.tensor.matmul(psA, X[:, M:256], wpsb[:, 130:260], start=False, stop=False)
                nc.tensor.matmul(psA, prod[:, 512:640], wpsb[:, 130:260], start=False, stop=False)
                nc.tensor.matmul(psA, prod[:, 640:768], wpsb[:, 130:260], start=False, stop=False)
                nc.tensor.matmul(psA, prod[:, 768:896], wpsb[:, 524:654], start=False, stop=False)
                nc.tensor.matmul(psA, prod[:, 896:1024], wpsb[:, 524:654], start=False, stop=True)
                t1sb = pw.tile([M, 130], BF16, tag="t1sb")
                nc.scalar.activation(t1sb, psA, AF.Copy)

                # stage B batched 3 slots per PSUM bank; one masked copy-out
                g, r = divmod(s, 3)
                if r == 0:
                    psB = pq.tile([65, 396], F32, tag="psB")
                pBs = psB[:, r * 132:(r + 1) * 132]
                nc.tensor.matmul(pBs, t1sb[:, 0:65], wpsb[:, 260:392], start=True, stop=False)
                nc.tensor.matmul(pBs, t1sb[:, 65:130], wpsb[:, 392:524], start=False, stop=True)
                if r == 2 or s == NPK - 1:
                    s0 = 3 * g
                    w = (r + 1) * 132
                    nc.vector.tensor_tensor(
                        OGB[:, 132 * s0:132 * s0 + w], psB[:, 0:w],
                        MG[:, 132 * s0:132 * s0 + w], OP.mult)

            # 4 big output DMAs: (rows 0..32 | 96..127) x (cols 0..32 | 95..127)
            ogp = OGB[:].rearrange("p (pair c v) -> p pair c v", c=2, v=33)
            for (r0, rn, po) in [(0, 33, 0), (33, 32, 96 * M)]:
                for (cj, co) in [(0, 0), (1, 95)]:
                    dst = bass.AP(outd, po + co,
                                  [[M, rn], [M * M, NPAIR], [1, 33]])
                    nc.sync.dma_start(dst, ogp[r0:r0 + rn, :, cj])

    nc.compile()
    return nc


def _host_tables(la1, la2, shifted):
    """Shared-la1 slot pairing: rows sorted into la1 buckets (padded to even
    size with discarded duplicates), buckets distributed over 4 quarters.
    Returns per-quarter (offs [1,NPK*3] int32, mask idx [NPAIR], perm [NPAIR]
    mapping output row -> global pair index, -1 for pads/dummies)."""
    la1 = np.asarray(la1, np.int64)
    la2 = np.asarray(la2, np.int64)
    sh = np.asarray(shifted, np.int64)
    P = la1.shape[0]
    rows = [(int(la1[i]), int(la2[i]), int(sh[i]), i) for i in range(P)]
    rows.append((80, 80, 5, P))        # phi autocorr pair
    buckets = {}
    for r in rows:
        buckets.setdefault(r[0], []).append(r)
    blist = []
    for v in sorted(buckets):
        b = buckets[v]
        if len(b) % 2:
            b.append((b[-1][0], b[-1][1], b[-1][2], -1))   # dummy partner
        blist.append(b)
    quarters = [[] for _ in range(4)]
    qi = 0
    for b in blist:
        while len(quarters[qi]) + len(b) > NPAIR:
            qi += 1
            if qi >= 4:
                raise RuntimeError("pair table too irregular for NPK slots")
        quarters[qi].extend(b)
    offs, mgi, perms = [], [], []
    for q in range(4):
        rs = quarters[q]
        o = np.zeros((NPK, 3), np.int32)
        mi = np.zeros(NPAIR, np.int64)
        pm = np.full(NPAIR, -1, np.int64)
        for i, (l1, l2, s_, g) in enumerate(rs):
            sl, half = divmod(i, 2)
            o[sl, 0] = l1 * 512
            o[sl, 1 + half] = l2 * 512
            mi[i] = s_
            pm[i] = g
        offs.append(o.reshape(1, -1))
        mgi.append(mi)
        perms.append(pm)
    return offs, mgi, perms


def _host_consts():
    k = np.arange(M)
    W = np.exp(-2j * np.pi * np.outer(k, k) / M)
    Wr = W.real.astype(np.float32)
    Wi = W.imag.astype(np.float32)
    V = np.conj(W) / M
    Ar = V.real.astype(np.float32)
    Ai = V.imag.astype(np.float32)
    Pr = Ar / 16384.0
    Pi = Ai / 16384.0
    blocks = [Wr, Wi, -Wi, Wr,            # 0-3: fwd stage1 + hatx stage2
              Ar, Ai, -Ai, Ar,            # 4-7: inverse stages
              Wr, Wi, Wi, -Wr,            # 8-11: fwd stage2 rhs1 (512)
              -Wi, Wr, Wr, Wi]            # 12-15: fwd stage2 rhs2 (512)
    wmat = np.concatenate(blocks, axis=1).astype(ml_dtypes.bfloat16)
    sa = np.r_[0:33, 96:128]
    sb = np.r_[0:33, 95:128]
    wpair = np.concatenate(
        [Ar[:, sa], Ai[:, sa], -Ai[:, sa], Ar[:, sa],
         Pr[:, sb], Pi[:, sb], -Pi[:, sb], Pr[:, sb],
         -Ar[:, sa], -Ai[:, sa]], axis=1
    ).astype(ml_dtypes.bfloat16)
    cns = np.zeros((M, 260), np.float32)
    cns[:, 0] = 1.0
    cns[:, 2:130] = np.eye(M, dtype=np.float32)
    cns[:, 130:258] = 1.0
    return wmat, wpair, cns


def _prepare(inputs):
    x = np.asarray(inputs["x"], np.float32)
    hatpsi = np.asarray(inputs["hatpsi"], np.float32)
    hatphi = np.asarray(inputs["hatphi"], np.float32)
    masks = np.asarray(inputs["masks_shift"], np.float32)
    la1 = np.asarray(inputs["la1"])
    la2 = np.asarray(inputs["la2"])
    shifted = np.asarray(inputs["shifted"])

    wmat, wpair, cns = _host_consts()
    psi = np.concatenate(
        [hatpsi.transpose(2, 0, 1, 3).reshape(M, 20 * M), hatphi], axis=1
    ).astype(ml_dtypes.bfloat16)
    offs, mgi, perms = _host_tables(la1, la2, shifted)
    _cache["perms"] = perms
    masks_bf = masks.astype(ml_dtypes.bfloat16)
    sa = np.r_[0:33, 96:128]
    sb = np.r_[0:33, 95:128]

    in_maps = []
    for c in range(8):
        b, q = c // 4, c % 4
        mg = masks_bf[mgi[q]]                       # [NPAIR,128,128]
        mg = mg[:, sa][:, :, sb]                    # [NPAIR,65,66]
        mg = np.ascontiguousarray(mg.transpose(1, 0, 2)).reshape(65, NPAIR * 66)
        in_maps.append({
            "xin": np.ascontiguousarray(x[b, 0]),
            "psi": psi,
            "wmat": wmat,
            "wpair": wpair,
            "mgath": mg,
            "cns": cns,
            "offs": offs[q],
        })
    return in_maps


def _assemble(results):
    out = np.empty((2, P_TOT, M * M), np.float32)
    perms = _cache["perms"]
    for c in range(8):
        b, q = c // 4, c % 4
        r = results[c]["out"].reshape(NPAIR, M * M)
        pm = perms[q]
        mvalid = pm >= 0
        out[b, pm[mvalid]] = r[mvalid].astype(np.float32)
    return out.reshape(2, -1)


def kernel(**inputs):
    if "nc" not in _cache:
        _cache["nc"] = _build_nc()
    nc = _cache["nc"]
    in_maps = _prepare(inputs)
    res = run_bass_kernel_spmd(nc, in_maps, core_ids=list(range(8)))
    return _assemble(res.results)


def kernel_traced(tmpdir=None, **inputs):
    """Like kernel() but with neuron-profile tracing; returns (out, results)."""
    if "nc" not in _cache:
        _cache["nc"] = _build_nc()
    nc = _cache["nc"]
    in_maps = _prepare(inputs)
    res = run_bass_kernel_spmd(
        nc, in_maps, core_ids=list(range(8)), trace=True, tmpdir=tmpdir
    )
    return _assemble(res.results), res


# revision 16
# speedup vs baseline: 1.2476x; 1.0445x over previous
"""ALPHA scattering-covariance kernel for 8 Trainium2 NeuronCores.

Math (per batch element b, nc=1, M=N=128, J=5 L=4 A=4, P=960 pairs + 1 phi):
  hatx = fft2(x)
  z_jl = ifft2(hatx * hatpsi[j,l])           (20 complex fields)
  field k=4*jl+a: u = relu(phase_a(z_jl)),   phase in {Re, -Im, -Re, +Im}
  phi field (k=80): u = Re(ifft2(hatx*hatphi))   (no relu)
  n_k = (u - mean)/std ;  hat_n_k = fft2(n_k)
  pair p: corr = Re(ifft2(hat_n[la1] * conj(hat_n[la2]))) / (M*N) * masks[shifted]
  out[b] = concat(pairs 0..959, phi autocorr)

2D FFTs run as transpose-matmuls on the TensorEngine: tmm(P, Q) := P^T @ Q with
P stationary; fft2(n) = W n W (W symmetric), two chained tmms absorb the
inter-stage transposes. Pairs are processed two-per-slot packed into one
complex IFFT (both spectra Hermitian, so ifft2(H_p + i H_q) = corr_p + i corr_q).

Normalization is folded into the spectral domain: fft2((u-mu)/std) =
fft2(u)/std with the DC coefficient forced to 0 (mean-sub only affects DC).
So the raw field FFTs run unscaled, the PSUM->SBUF copy applies 1/std, and a
single strided memset zeroes the DC entries of all 81 spectra.

The per-pair combine Zr = Hr_p - Hi_q, Zi = Hi_p + Hr_q is absorbed into the
first IFFT stage's PSUM accumulation (4 matmuls against +/- copies of the
stage-A weights) instead of DVE ops -- DVE is the pair-loop bottleneck.

Sharding: core c = batch (c//4) x pair-quarter (c%4). Spectra of all 81 fields
live in SBUF as (re|im|im|-re) 512-blocks; per-pair operands are fetched with
register-offset dynamic APs (offsets la*512 from an int32 input table), so one
static graph serves all 8 cores.
"""

import numpy as np
import ml_dtypes

import concourse.bass as bass
import concourse.bacc as bacc
import concourse.tile as tile
import concourse.mybir as mybir
from concourse.bass_utils import run_bass_kernel_spmd

BF16 = mybir.dt.bfloat16
F32 = mybir.dt.float32
I32 = mybir.dt.int32
AF = mybir.ActivationFunctionType
OP = mybir.AluOpType
DVE = mybir.EngineType.DVE

M = 128
NJL = 21          # 20 (j,l) filters + phi as slot 20
NFLD = 81         # 80 alpha fields + normalized phi field as field 80
NPK = 121         # packed pair slots per core (2 pairs each)
NPAIR = 242       # padded pairs per core
QUARTS = [(0, 241), (241, 240), (481, 240), (721, 240)]  # (start, count) of 961
P_TOT = 961

_cache = {}


def _build_nc():
    nc = bacc.Bacc("TRN2", target_bir_lowering=False, debug=False, num_devices=8)

    xin = nc.dram_tensor("xin", [M, M], F32, kind="ExternalInput")
    psid = nc.dram_tensor("psi", [M, NJL * M], BF16, kind="ExternalInput")
    wmatd = nc.dram_tensor("wmat", [M, 16 * M], BF16, kind="ExternalInput")
    wpaird = nc.dram_tensor("wpair", [M, 654], BF16, kind="ExternalInput")
    mgd = nc.dram_tensor("mgath", [65, NPK * 132], BF16, kind="ExternalInput")
    cnsd = nc.dram_tensor("cns", [M, 260], F32, kind="ExternalInput")
    offsd = nc.dram_tensor("offs", [1, NPK * 3], I32, kind="ExternalInput")
    outd = nc.dram_tensor("out", [NPAIR, M, M], BF16, kind="ExternalOutput")

    with tile.TileContext(nc) as tc:
        with (
            tc.tile_pool(name="const", bufs=1) as cp,
            tc.tile_pool(name="work", bufs=3) as wp,
            tc.tile_pool(name="pairw", bufs=4) as pw,
            tc.tile_pool(name="ps256", bufs=2, space="PSUM") as pp,
            tc.tile_pool(name="ps512", bufs=2, space="PSUM") as pv,
            tc.tile_pool(name="psB", bufs=4, space="PSUM") as pq,
        ):
            # ---- constants into SBUF ----
            wsb = cp.tile([M, 16 * M], BF16)
            nc.sync.dma_start(wsb, wmatd.ap())
            wpsb = cp.tile([M, 654], BF16)
            nc.sync.dma_start(wpsb, wpaird.ap())
            psisb = cp.tile([M, NJL * M], BF16)
            nc.sync.dma_start(psisb, psid.ap())
            MG = cp.tile([65, NPK * 132], BF16)
            nc.sync.dma_start(MG, mgd.ap())
            cns = cp.tile([M, 260], F32)
            nc.sync.dma_start(cns, cnsd.ap())
            offsb = cp.tile([1, NPK * 3], I32)
            nc.sync.dma_start(offsb, offsd.ap())
            xf = cp.tile([M, M], F32)
            nc.sync.dma_start(xf, xin.ap())

            # zero-fill the never-written regions of out: mid rows 33..95 and
            # dead cols 33..94 of the live rows. Few big DMAs, 4 queue-chunks.
            zsb = cp.tile([M, M], BF16)
            nc.vector.memzero(zsb)
            for j0 in range(0, NPAIR, 61):
                n = min(61, NPAIR - j0)
                # rows 33..95 full width
                dst = bass.AP(outd, j0 * M * M + 33 * M,
                              [[M, 63], [M * M, n], [1, M]])
                src = zsb[0:63, 0:M].rearrange("p (one x) -> p one x", one=1)
                nc.sync.dma_start(dst, src.to_broadcast((63, n, M)))
                # rows 0..32, cols 33..94
                dst = bass.AP(outd, j0 * M * M + 33,
                              [[M, 33], [M * M, n], [1, 62]])
                src = zsb[0:33, 0:62].rearrange("p (one x) -> p one x", one=1)
                nc.sync.dma_start(dst, src.to_broadcast((33, n, 62)))
                # rows 96..127, cols 33..94
                dst = bass.AP(outd, j0 * M * M + 96 * M + 33,
                              [[M, 32], [M * M, n], [1, 62]])
                src = zsb[0:32, 0:62].rearrange("p (one x) -> p one x", one=1)
                nc.sync.dma_start(dst, src.to_broadcast((32, n, 62)))

            # big bf16 staging buffer for all live-window outputs
            OGB = cp.tile([65, NPK * 132], BF16)

            def WB(i, n=1):
                return wsb[:, i * M:(i + n) * M]

            ones_col = cns[:, 0:1]          # [128,1] f32 of 1.0
            ones_row = cns[0:1, 130:258]    # [1,128] f32 of 1.0
            ident = cns[:, 2:130]           # [128,128] f32 identity

            # persistent SBUF state
            SPBIG = cp.tile([M, NFLD * 512], BF16)  # (re|im|im|-re) per field
            FLDS = cp.tile([M, NFLD * M], BF16)     # relu'd fields
            HX = cp.tile([M, 256], BF16)            # hatx (re|im)
            SUMS = cp.tile([M, NFLD], F32)
            SQS = cp.tile([M, NFLD], F32)
            AB = cp.tile([M, NFLD], F32)            # per-field 1/std bcast

            # ---- hatx = fft2(x) ----
            xb = wp.tile([M, M], BF16, tag="xb")
            nc.scalar.activation(xb, xf, AF.Copy)
            psF = pp.tile([M, 256], F32, tag="ps256")
            nc.tensor.matmul(psF, xb, WB(0, 2), start=True, stop=True)
            fsb = wp.tile([M, 256], BF16, tag="fsb")
            nc.scalar.activation(fsb, psF, AF.Copy)
            psH = pp.tile([M, 256], F32, tag="ps256")
            nc.tensor.matmul(psH, fsb[:, 0:M], WB(0, 2), start=True, stop=False)
            nc.tensor.matmul(psH, fsb[:, M:256], WB(2, 2), start=False, stop=True)
            nc.scalar.activation(HX, psH, AF.Copy)

            # ---- z_jl = ifft2(hatx * psi_jl); fields + row-sums ----
            HX3 = HX[:, 0:256].rearrange("p (two c) -> p two c", two=2)
            for jl in range(NJL):
                pj = psisb[:, jl * M:(jl + 1) * M]
                pj3 = pj.rearrange("p (one c) -> p one c", one=1)
                ab2 = wp.tile([M, 256], BF16, tag="ab2")
                nc.vector.tensor_tensor(
                    ab2.rearrange("p (two c) -> p two c", two=2),
                    HX3, pj3.to_broadcast((M, 2, M)), OP.mult)
                psT = pp.tile([M, 256], F32, tag="ps256")
                nc.tensor.matmul(psT, ab2[:, 0:M], WB(4, 2), start=True, stop=False)
                nc.tensor.matmul(psT, ab2[:, M:256], WB(6, 2), start=False, stop=True)
                tsb = wp.tile([M, 256], BF16, tag="tsb")
                if jl % 2 == 0:
                    nc.scalar.activation(tsb, psT, AF.Copy)
                else:
                    nc.vector.tensor_scalar(tsb, psT, 1.0, None, OP.mult)
                psZ = pp.tile([M, 256], F32, tag="ps256")
                nc.tensor.matmul(psZ, tsb[:, 0:M], WB(4, 2), start=True, stop=False)
                nc.tensor.matmul(psZ, tsb[:, M:256], WB(6, 2), start=False, stop=True)
                if jl < 20:
                    # fields 4jl+a: relu(+zr), relu(-zi), relu(-zr), relu(+zi)
                    # a=0,1 on ACT (relu + fused row-sum), a=2,3 on DVE
                    for a, (half, sc) in enumerate(
                        [(0, 1.0), (1, -1.0), (0, -1.0), (1, 1.0)]
                    ):
                        k = 4 * jl + a
                        fld = FLDS[:, k * M:(k + 1) * M]
                        src = psZ[:, half * M:(half + 1) * M]
                        if a < 2:
                            nc.scalar.activation(
                                fld, src, AF.Relu,
                                scale=sc, accum_out=SUMS[:, k:k + 1],
                            )
                        else:
                            nc.vector.tensor_scalar(
                                fld, src, sc, 0.0, OP.mult, OP.max)
                            nc.vector.tensor_reduce(
                                SUMS[:, k:k + 1], fld, mybir.AxisListType.X, OP.add)
                    # squared sums: a=0,1 on ACT Square, a=2,3 on DVE
                    for a in range(4):
                        k = 4 * jl + a
                        fld = FLDS[:, k * M:(k + 1) * M]
                        dump = wp.tile([M, M], BF16, tag="dump")
                        if a < 2:
                            nc.scalar.activation(
                                dump, fld, AF.Square, accum_out=SQS[:, k:k + 1])
                        else:
                            nc.gpsimd.tensor_tensor(dump, fld, fld, OP.mult)
                            nc.vector.tensor_reduce(
                                SQS[:, k:k + 1], dump, mybir.AxisListType.X, OP.add)
                else:
                    k = 80
                    fld = FLDS[:, k * M:(k + 1) * M]
                    nc.scalar.activation(
                        fld, psZ[:, 0:M], AF.Copy,
                        accum_out=SUMS[:, k:k + 1],
                    )
                    dump = wp.tile([M, M], BF16, tag="dump")
                    nc.scalar.activation(
                        dump, fld, AF.Square, accum_out=SQS[:, k:k + 1])

            # ---- stats: total mean/var -> alpha = 1/std, bcast to [128, 81] ----
            psS = pp.tile([M, 256], F32, tag="ps256")
            nc.tensor.matmul(psS[0:NFLD, 0:1], SUMS, ones_col, start=True, stop=True)
            nc.tensor.matmul(psS[0:NFLD, 1:2], SQS, ones_col, start=True, stop=True)
            st = wp.tile([NFLD, 2], F32, tag="st1")
            nc.scalar.activation(st, psS[0:NFLD, 0:2], AF.Copy, scale=1.0 / 16384.0)
            mu = st[:, 0:1]
            e2 = st[:, 1:2]
            mu2 = wp.tile([NFLD, 1], F32, tag="mu2")
            nc.vector.tensor_tensor(mu2, mu, mu, OP.mult)
            var = wp.tile([NFLD, 1], F32, tag="var")
            nc.vector.tensor_tensor(var, e2, mu2, OP.subtract)
            alph = wp.tile([NFLD, 1], F32, tag="alph")
            sdev = wp.tile([NFLD, 1], F32, tag="sdev")
            nc.scalar.activation(sdev, var, AF.Sqrt)
            nc.vector.reciprocal(alph, sdev)
            psS2 = pp.tile([M, 256], F32, tag="ps256")
            nc.tensor.transpose(psS2[0:1, 0:NFLD], alph, ident[0:NFLD, 0:NFLD])
            arow = wp.tile([1, NFLD], F32, tag="arow")
            nc.scalar.activation(arow, psS2[0:1, 0:NFLD], AF.Copy)
            psAB = pp.tile([M, 256], F32, tag="ps256")
            nc.tensor.matmul(psAB[:, 0:NFLD], ones_row, arow, start=True, stop=True)
            nc.scalar.activation(AB, psAB[:, 0:NFLD], AF.Copy)

            # ---- forward fft per field (unnormalized) -> SPBIG * (1/std) ----
            for k in range(NFLD):
                fld = FLDS[:, k * M:(k + 1) * M]
                psF2 = pp.tile([M, 256], F32, tag="ps256")
                nc.tensor.matmul(psF2, fld, WB(0, 2), start=True, stop=True)
                f2 = wp.tile([M, 256], BF16, tag="fsb")
                if k % 2 == 0:
                    nc.scalar.activation(f2, psF2, AF.Copy)
                else:
                    nc.vector.tensor_scalar(f2, psF2, 1.0, None, OP.mult)
                psH5 = pv.tile([M, 512], F32, tag="ps512")
                nc.tensor.matmul(psH5, f2[:, 0:M], WB(8, 4), start=True, stop=False)
                nc.tensor.matmul(psH5, f2[:, M:256], WB(12, 4), start=False, stop=True)
                dst = SPBIG[:, k * 512:(k + 1) * 512]
                if k % 2 == 0:
                    nc.vector.tensor_scalar(dst, psH5, AB[:, k:k + 1], None, OP.mult)
                else:
                    nc.scalar.activation(dst, psH5, AF.Copy, scale=AB[:, k:k + 1])
            # mean-sub == zero the DC entry of every spectrum (all 4 sub-blocks)
            spz = SPBIG[:].rearrange("p (k c) -> p k c", c=512)
            for off in range(0, 512, 128):
                nc.vector.memset(spz[0:1, :, off:off + 1], 0.0)

            # ---- packed pair loop: slot s covers pairs (2s, 2s+1),
            # host-paired so both share la1 (one lhs offset per slot) ----
            vals = None
            psB = None
            for s in range(NPK):
                if s % 8 == 0:
                    nv = min(8, NPK - s) * 3
                    _, vals = nc.values_load_multi_w_load_instructions(
                        offsb[0:1, 3 * s:3 * s + nv],
                        engines=[DVE],
                        min_val=0, max_val=80 * 512,
                        skip_runtime_bounds_check=True,
                    )
                o1, o2p, o2q = vals[3 * (s % 8):3 * (s % 8) + 3]

                # products: (t1|t2|t3|-t4) per pair; lhs is the (re|im|im|-re)
                # block of la1 (shared), rhs the (re|im) half of la2 twice
                prod = pw.tile([M, 1024], BF16, tag="prod")
                rhs_p = SPBIG[:, bass.ds(o2p, 256)].rearrange(
                    "p (one c) -> p one c", one=1).to_broadcast((M, 2, 256))
                rhs_q = SPBIG[:, bass.ds(o2q, 256)].rearrange(
                    "p (one c) -> p one c", one=1).to_broadcast((M, 2, 256))
                lhs = SPBIG[:, bass.ds(o1, 512)].rearrange(
                    "p (two c) -> p two c", two=2)
                nc.vector.tensor_tensor(
                    prod[:, 0:512].rearrange("p (two c) -> p two c", two=2),
                    lhs, rhs_p, OP.mult)
                nc.vector.tensor_tensor(
                    prod[:, 512:1024].rearrange("p (two c) -> p two c", two=2),
                    lhs, rhs_q, OP.mult)

                # X = (Hr_p|Hi_p) for pair p only; pair q's four product
                # blocks go straight into the psA accumulation below
                X = pw.tile([M, 256], BF16, tag="X")
                pr3 = prod[:, 0:512].rearrange("p (n two c) -> p n two c", two=2, c=M)
                xeng = nc.gpsimd if s % 8 != 1 else nc.vector
                xeng.tensor_tensor(
                    X[:].rearrange("p (n c) -> p n c", c=M),
                    pr3[:, :, 0, :], pr3[:, :, 1, :], OP.add)

                # packed ifft stage A; all +/- combines folded into PSUM:
                # psA = Hr_p w0 + Hi_p w1 + (t1q+t2q) w1 + (t3q+t4mq)(-w0).
                # prod-fed matmuls run first so the (slower, gpsimd) X add
                # overlaps them; two slots share one PSUM bank for depth.
                ga, ra = divmod(s, 2)
                if ra == 0:
                    psA = pv.tile([M, 260], F32, tag="ps512",
                                  padded_shape=[M, 512])
                    t1sb = pw.tile([M, 260], BF16, tag="t1sb")
                pAs = psA[:, ra * 130:(ra + 1) * 130]
                nc.tensor.matmul(pAs, prod[:, 512:640], wpsb[:, 130:260], start=True, stop=False)
                nc.tensor.matmul(pAs, prod[:, 640:768], wpsb[:, 130:260], start=False, stop=False)
                nc.tensor.matmul(pAs, prod[:, 768:896], wpsb[:, 524:654], start=False, stop=False)
                nc.tensor.matmul(pAs, prod[:, 896:1024], wpsb[:, 524:654], start=False, stop=False)
                nc.tensor.matmul(pAs, X[:, 0:M], wpsb[:, 0:130], start=False, stop=False)
                nc.tensor.matmul(pAs, X[:, M:256], wpsb[:, 130:260], start=False, stop=True)
                if ra == 1 or s == NPK - 1:
                    nc.scalar.activation(t1sb[:, 0:(ra + 1) * 130],
                                         psA[:, 0:(ra + 1) * 130], AF.Copy)
                    # stage B (3 slots per PSUM bank) for the evacuated slots
                    for sl in range(2 * ga, s + 1):
                        off = (sl - 2 * ga) * 130
                        g, r = divmod(sl, 3)
                        if r == 0:
                            psB = pq.tile([65, 396], F32, tag="psB")
                        pBs = psB[:, r * 132:(r + 1) * 132]
                        nc.tensor.matmul(pBs, t1sb[:, off:off + 65],
                                         wpsb[:, 260:392], start=True, stop=False)
                        nc.tensor.matmul(pBs, t1sb[:, off + 65:off + 130],
                                         wpsb[:, 392:524], start=False, stop=True)
                        if r == 2 or sl == NPK - 1:
                            s0 = 3 * g
                            w = (r + 1) * 132
                            nc.vector.tensor_tensor(
                                OGB[:, 132 * s0:132 * s0 + w], psB[:, 0:w],
                                MG[:, 132 * s0:132 * s0 + w], OP.mult)

            # 4 big output DMAs: (rows 0..32 | 96..127) x (cols 0..32 | 95..127)
            ogp = OGB[:].rearrange("p (pair c v) -> p pair c v", c=2, v=33)
            for (r0, rn, po) in [(0, 33, 0), (33, 32, 96 * M)]:
                for (cj, co) in [(0, 0), (1, 95)]:
                    dst = bass.AP(outd, po + co,
                                  [[M, rn], [M * M, NPAIR], [1, 33]])
                    nc.sync.dma_start(dst, ogp[r0:r0 + rn, :, cj])

    nc.compile()
    return nc


def _host_tables(la1, la2, shifted):
    """Shared-la1 slot pairing: rows sorted into la1 buckets (padded to even
    size with discarded duplicates), buckets distributed over 4 quarters.
    Returns per-quarter (offs [1,NPK*3] int32, mask idx [NPAIR], perm [NPAIR]
    mapping output row -> global pair index, -1 for pads/dummies)."""
    la1 = np.asarray(la1, np.int64)
    la2 = np.asarray(la2, np.int64)
    sh = np.asarray(shifted, np.int64)
    P = la1.shape[0]
    rows = [(int(la1[i]), int(la2[i]), int(sh[i]), i) for i in range(P)]
    rows.append((80, 80, 5, P))        # phi autocorr pair
    buckets = {}
    for r in rows:
        buckets.setdefault(r[0], []).append(r)
    blist = []
    for v in sorted(buckets):
        b = buckets[v]
        if len(b) % 2:
            b.append((b[-1][0], b[-1][1], b[-1][2], -1))   # dummy partner
        blist.append(b)
    quarters = [[] for _ in range(4)]
    qi = 0
    for b in blist:
        while len(quarters[qi]) + len(b) > NPAIR:
            qi += 1
            if qi >= 4:
                raise RuntimeError("pair table too irregular for NPK slots")
        quarters[qi].extend(b)
    offs, mgi, perms = [], [], []
    for q in range(4):
        rs = quarters[q]
        o = np.zeros((NPK, 3), np.int32)
        mi = np.zeros(NPAIR, np.int64)
        pm = np.full(NPAIR, -1, np.int64)
        for i, (l1, l2, s_, g) in enumerate(rs):
            sl, half = divmod(i, 2)
            o[sl, 0] = l1 * 512
            o[sl, 1 + half] = l2 * 512
            mi[i] = s_
            pm[i] = g
        offs.append(o.reshape(1, -1))
        mgi.append(mi)
        perms.append(pm)
    return offs, mgi, perms


def _host_consts():
    k = np.arange(M)
    W = np.exp(-2j * np.pi * np.outer(k, k) / M)
    Wr = W.real.astype(np.float32)
    Wi = W.imag.astype(np.float32)
    V = np.conj(W) / M
    Ar = V.real.astype(np.float32)
    Ai = V.imag.astype(np.float32)
    Pr = Ar / 16384.0
    Pi = Ai / 16384.0
    blocks = [Wr, Wi, -Wi, Wr,            # 0-3: fwd stage1 + hatx stage2
              Ar, Ai, -Ai, Ar,            # 4-7: inverse stages
              Wr, Wi, Wi, -Wr,            # 8-11: fwd stage2 rhs1 (512)
              -Wi, Wr, Wr, Wi]            # 12-15: fwd stage2 rhs2 (512)
    wmat = np.concatenate(blocks, axis=1).astype(ml_dtypes.bfloat16)
    sa = np.r_[0:33, 96:128]
    sb = np.r_[0:33, 95:128]
    wpair = np.concatenate(
        [Ar[:, sa], Ai[:, sa], -Ai[:, sa], Ar[:, sa],
         Pr[:, sb], Pi[:, sb], -Pi[:, sb], Pr[:, sb],
         -Ar[:, sa], -Ai[:, sa]], axis=1
    ).astype(ml_dtypes.bfloat16)
    cns = np.zeros((M, 260), np.float32)
    cns[:, 0] = 1.0
    cns[:, 2:130] = np.eye(M, dtype=np.float32)
    cns[:, 130:258] = 1.0
    return wmat, wpair, cns


def _prepare(inputs):
    x = np.asarray(inputs["x"], np.float32)
    hatpsi = np.asarray(inputs["hatpsi"], np.float32)
    hatphi = np.asarray(inputs["hatphi"], np.float32)
    masks = np.asarray(inputs["masks_shift"], np.float32)
    la1 = np.asarray(inputs["la1"])
    la2 = np.asarray(inputs["la2"])
    shifted = np.asarray(inputs["shifted"])

    wmat, wpair, cns = _host_consts()
    psi = np.concatenate(
        [hatpsi.transpose(2, 0, 1, 3).reshape(M, 20 * M), hatphi], axis=1
    ).astype(ml_dtypes.bfloat16)
    offs, mgi, perms = _host_tables(la1, la2, shifted)
    _cache["perms"] = perms
    masks_bf = masks.astype(ml_dtypes.bfloat16)
    sa = np.r_[0:33, 96:128]
    sb = np.r_[0:33, 95:128]

    in_maps = []
    for c in range(8):
        b, q = c // 4, c % 4
        mg = masks_bf[mgi[q]]                       # [NPAIR,128,128]
        mg = mg[:, sa][:, :, sb]                    # [NPAIR,65,66]
        mg = np.ascontiguousarray(mg.transpose(1, 0, 2)).reshape(65, NPAIR * 66)
        in_maps.append({
            "xin": np.ascontiguousarray(x[b, 0]),
            "psi": psi,
            "wmat": wmat,
            "wpair": wpair,
            "mgath": mg,
            "cns": cns,
            "offs": offs[q],
        })
    return in_maps


def _assemble(results):
    out = np.empty((2, P_TOT, M * M), np.float32)
    perms = _cache["perms"]
    for c in range(8):
        b, q = c // 4, c % 4
        r = results[c]["out"].reshape(NPAIR, M * M)
        pm = perms[q]
        mvalid = pm >= 0
        out[b, pm[mvalid]] = r[mvalid].astype(np.float32)
    return out.reshape(2, -1)


def kernel(**inputs):
    if "nc" not in _cache:
        _cache["nc"] = _build_nc()
    nc = _cache["nc"]
    in_maps = _prepare(inputs)
    res = run_bass_kernel_spmd(nc, in_maps, core_ids=list(range(8)))
    return _assemble(res.results)


def kernel_traced(tmpdir=None, **inputs):
    """Like kernel() but with neuron-profile tracing; returns (out, results)."""
    if "nc" not in _cache:
        _cache["nc"] = _build_nc()
    nc = _cache["nc"]
    in_maps = _prepare(inputs)
    res = run_bass_kernel_spmd(
        nc, in_maps, core_ids=list(range(8)), trace=True, tmpdir=tmpdir
    )
    return _assemble(res.results), res
